# revision 13
# baseline (speedup 1.0000x reference)
"""Trainium2 Bass kernel for nn_NodeModel (GNN message passing + 3-layer node MLP).

Strategy (node-parallel, 8 cores), v2 — transpose-free [h, node] dataflow:
  - Host: sort edges by destination node, bucket into 128-node tiles, pad each
    tile's edge list to K chunks of 128 edges. Nodes sharded contiguously
    across 8 cores (12800 padded nodes each, 25 super-tiles of 512).
  - LayerNorm mean is folded into the weights on host (W' = W - rowmean(W),
    b' = b - mean(b)) so every matmul output is already mean-centered.
  - Per 512-node super-tile on device:
      * aggregation: one-hot sel built by tensor_scalar(iota == col) on
        DVE/GPSIMD, chunk matmuls accumulate aggT[h, n] in PSUM.
      * per layer: bias seeded into PSUM by a rank-1 matmul (b'_row x ones),
        z_c = W'.T @ aT accumulated on top; var broadcast to all partitions
        via an all-ones stationary matmul of sq = z_c^2; rsig = exp(-.5*ln(.));
        zn = z_c * rsig; ssp via two ACT ops Exp(g*x+be), Ln(.5x+.5).
    Everything stays [h, node]; no PE transposes, no bn_stats.
  - ssp's -log2 appears exactly as ln(0.5 e^y + 0.5); output written bf16.
"""

import os
import sys

import numpy as np

sys.path.insert(0, "/opt/trn_rl_repo")

import bass_rust as _bass_rust
import ml_dtypes

from concourse import bacc, bass, hw_specs, mybir
from concourse import tile as tile_mod
from concourse.bass_utils import run_bass_kernel_spmd


class _Bacc(bacc.Bacc):
    """Bacc with the ACT table chooser pinned to the single function set
    that holds Ln+Exp+Copy+Identity. The default greedy chooser alternates
    between per-func sets, costing a ~1.3us ACT_TABLE_LOAD per switch."""

    def insert_act_table_loads(self):
        has_activation = any(
            isinstance(i, mybir.InstActivation)
            for b in self.main_func.blocks
            for i in b.instructions
        )
        if not has_activation:
            return
        keep = "natural_log_exp_and_others"
        tables = [
            (n, (s if n == keep else set()))
            for n, s in hw_specs.get_activation_tables(self.m.arch).items()
        ]
        _bass_rust.insert_act_table_loads(self, tables)


LOG2 = float(np.log(2.0))
N, E, H = 100000, 600000, 128
NC = 8
P = 128
SN = 512                 # nodes per super-tile
TPS = SN // P            # 128-node tiles per super-tile (4)
SPC = 25                 # super-tiles per core
TPC = SPC * TPS          # 128-node tiles per core (100)
NPC = TPC * P            # nodes per core (12800)
NPAD = NPC * NC          # padded node count (102400)
NT = NPAD // P           # total node tiles (800)

F32 = mybir.dt.float32
BF16 = mybir.dt.bfloat16

LAST_RESULT = None  # BassKernelResults of the most recent run (for profiling)


def _host_prep(x, edge_index, edge_attr):
    col = np.asarray(edge_index)[1].astype(np.int64)
    ea = np.ascontiguousarray(np.asarray(edge_attr, dtype=np.float32))
    order = np.argsort(col, kind="stable")
    col_s = col[order]
    tile_of = col_s >> 7
    counts = np.bincount(tile_of, minlength=NT)
    K = int(np.ceil(counts.max() / P))
    S = K * P
    starts = np.zeros(NT + 1, np.int64)
    starts[1:] = np.cumsum(counts)
    pos = np.arange(E) - starts[tile_of]
    slot = tile_of * S + pos
    slot_edge = np.zeros(NT * S, np.int64)
    slot_edge[slot] = order
    col_local = np.full(NT * S, 128.0, np.float32)
    col_local[slot] = (col_s & 127).astype(np.float32)
    payload = ea[slot_edge]  # [NT*S, H]

    x_pad = np.zeros((NPAD, H), np.float32)
    x_pad[:N] = np.asarray(x, dtype=np.float32)

    per_core = []
    for c in range(NC):
        r0, r1 = c * TPC * S, (c + 1) * TPC * S
        # ed rows = edge position within chunk, cols = (tile-chunk, h)
        pay_c = np.ascontiguousarray(
            payload[r0:r1]
            .reshape(TPC, K, P, H)
            .transpose(0, 2, 1, 3)
            .reshape(TPC * P, K * H)
            .astype(ml_dtypes.bfloat16)
        )
        col_c = np.ascontiguousarray(
            col_local[r0:r1].reshape(TPC, K, P).transpose(2, 0, 1).reshape(P, TPC * K)
        )
        # xT: [h, node] per core
        xt_c = np.ascontiguousarray(
            x_pad[c * NPC : (c + 1) * NPC].T.astype(ml_dtypes.bfloat16)
        )
        per_core.append((pay_c, col_c, xt_c))
    return K, per_core


def _build_program(K):
    # Bacc (not raw Bass): its compile pass splits multi-semaphore waits into
    # event-semaphore chains — walrus codegen allows only 1 wait per
    # instruction on this toolchain.
    nc = _Bacc("TRN2", target_bir_lowering=False, debug=False, num_devices=NC)

    edges_h = nc.dram_tensor("edges", [TPC * P, K * P], BF16, kind="ExternalInput")
    cols_h = nc.dram_tensor("cols", [P, TPC * K], F32, kind="ExternalInput")
    xt_h = nc.dram_tensor("xt", [P, NPC], BF16, kind="ExternalInput")
    w_h = {
        name: nc.dram_tensor(name, [P, P], BF16, kind="ExternalInput")
        for name in ("w1a", "w1b", "w2", "w3")
    }
    # bias rows b1',b2',b3' (already mean-centered) as [1,128] for rank-1 seed
    brow_h = {
        i: nc.dram_tensor(f"b{i}", [1, P], BF16, kind="ExternalInput")
        for i in (1, 2, 3)
    }
    # g1..g3, be1..be3 packed as columns of one tensor
    vecs_h = nc.dram_tensor("vecs", [P, 6], F32, kind="ExternalInput")
    # iota ramp tiled K times along free dim, for wide one-hot builds
    iota_h = nc.dram_tensor("iota", [P, K * P], F32, kind="ExternalInput")
    out_h = nc.dram_tensor("out", [P, NPC], BF16, kind="ExternalOutput")
    VIDX = {n: i for i, n in enumerate(("g1", "g2", "g3", "be1", "be2", "be3"))}

    n_st = int(os.environ.get("KERNEL_SPC", str(SPC)))

    with tile_mod.TileContext(nc) as tc:
        with (
            tc.tile_pool(name="const", bufs=1) as cpool,
            tc.tile_pool(name="edges", bufs=10) as epool,
            tc.tile_pool(name="xin", bufs=6) as xpool,
            tc.tile_pool(name="sel", bufs=8) as selpool,
            tc.tile_pool(name="aggs", bufs=6) as apool,
            tc.tile_pool(name="sq", bufs=4) as sqpool,
            tc.tile_pool(name="rs", bufs=4) as rspool,
            tc.tile_pool(name="zn", bufs=4) as znpool,
            tc.tile_pool(name="ez", bufs=4) as ezpool,
            tc.tile_pool(name="hout", bufs=6) as hpool,
            # PSUM budget (8 banks): 4 agg + 2 z + 2 var. z/var are freed
            # immediately by their single reader, agg gates ST pipelining.
            tc.tile_pool(name="psagg", bufs=4, space="PSUM") as pagg,
            tc.tile_pool(name="psz", bufs=2, space="PSUM") as psz,
            tc.tile_pool(name="psvar", bufs=2, space="PSUM") as psvar,
        ):
            iota = cpool.tile_from(iota_h[:])
            cols = cpool.tile_from(cols_h[:])
            W = {k: cpool.tile_from(h[:], name=f"w_{k}") for k, h in w_h.items()}
            brow = {i: cpool.tile_from(h[:], name=f"b_{i}") for i, h in brow_h.items()}
            vecs = cpool.tile_from(vecs_h[:])
            V = {n: vecs[:, i : i + 1] for n, i in VIDX.items()}
            ones_rep = cpool.tile([P, P], BF16)
            nc.gpsimd.memset(ones_rep[:], 1.0)
            ones_row = cpool.tile([1, SN], BF16)
            nc.gpsimd.memset(ones_row[:], 1.0)
            eps = cpool.tile([P, 1], F32)
            nc.gpsimd.memset(eps[:], 1e-5)
            half = cpool.tile([P, 1], F32)
            nc.gpsimd.memset(half[:], 0.5)

            for st in range(n_st):
                xT = xpool.tile([P, SN], BF16, tag="xT")
                nc.sync.dma_start(out=xT[:], in_=xt_h[:, st * SN : (st + 1) * SN])

                agg = pagg.tile([P, SN], F32, tag="agg")
                for t4 in range(TPS):
                    t = st * TPS + t4
                    ed = epool.tile([P, K * P], BF16, tag="ed")
                    nc.sync.dma_start(out=ed[:], in_=edges_h[t * P : (t + 1) * P, :])
                    # one wide one-hot build per tile: sel[e, k*128+n] =
                    # (cols[e, t*K+k] == n), via 3D broadcast APs
                    sel = selpool.tile([P, K * P], BF16, tag="sel")
                    # is_equal only lowers on DVE (Pool rejects compare ops)
                    nc.vector.tensor_tensor(
                        sel[:].rearrange("p (k n) -> p k n", k=K),
                        cols[:, t * K : (t + 1) * K].to_broadcast([P, K, P]),
                        iota[:].rearrange("p (k n) -> p k n", k=K),
                        op=mybir.AluOpType.is_equal,
                    )
                    for k in range(K):
                        nc.tensor.matmul(
                            out=agg[:, t4 * P : (t4 + 1) * P],
                            lhsT=ed[:, k * P : (k + 1) * P],
                            rhs=sel[:, k * P : (k + 1) * P],
                            start=(k == 0),
                            stop=(k == K - 1),
                        )
                aggS = apool.tile([P, SN], BF16, tag="aggS")
                nc.vector.tensor_copy(aggS[:], agg[:])

                aT = None
                for L in (1, 2, 3):
                    z = psz.tile([P, SN], F32, tag="z")
                    nc.tensor.matmul(
                        out=z[:], lhsT=brow[L][:], rhs=ones_row[:],
                        start=True, stop=False,
                    )
                    if L == 1:
                        nc.tensor.matmul(
                            out=z[:], lhsT=W["w1a"][:], rhs=xT[:],
                            start=False, stop=False,
                        )
                        nc.tensor.matmul(
                            out=z[:], lhsT=W["w1b"][:], rhs=aggS[:],
                            start=False, stop=True,
                        )
                    else:
                        nc.tensor.matmul(
                            out=z[:], lhsT=W[f"w{L}"][:], rhs=aT[:],
                            start=False, stop=True,
                        )
                    # PSUM has one DVE read port: copy z to SBUF bf16 first,
                    # then square in 2x bf16 mode.
                    zc = sqpool.tile([P, SN], BF16, tag="zc")
                    nc.vector.tensor_copy(zc[:], z[:])
                    # square on GPSIMD (SBUF-only engine) to offload DVE
                    sq = sqpool.tile([P, SN], BF16, tag="sq")
                    nc.gpsimd.tensor_tensor(
                        sq[:], zc[:], zc[:], op=mybir.AluOpType.mult
                    )
                    var = psvar.tile([P, SN], F32, tag="var")
                    nc.tensor.matmul(
                        out=var[:], lhsT=ones_rep[:], rhs=sq[:],
                        start=True, stop=True,
                    )
                    # rsig = exp(-0.5 * ln(var/H + eps)) broadcast over h
                    lnv = rspool.tile([P, SN], F32, tag="lnv")
                    nc.scalar.activation(
                        lnv[:], var[:], mybir.ActivationFunctionType.Ln,
                        bias=eps[:, 0:1], scale=1.0 / H,
                    )
                    rsig = rspool.tile([P, SN], BF16, tag="rsig")
                    nc.scalar.activation(
                        rsig[:], lnv[:], mybir.ActivationFunctionType.Exp,
                        scale=-0.5,
                    )
                    # zn on GPSIMD (all-SBUF operands) to offload DVE
                    zn = znpool.tile([P, SN], BF16, tag="zn")
                    nc.gpsimd.tensor_tensor(
                        zn[:], zc[:], rsig[:], op=mybir.AluOpType.mult
                    )
                    # ssp(y) = ln(0.5*exp(y) + 0.5), y = g*zn + be; includes
                    # the -log2 shift. |zn| <= sqrt(127) so exp cannot overflow.
                    ez = ezpool.tile([P, SN], F32, tag="ez")
                    nc.scalar.activation(
                        ez[:], zn[:], mybir.ActivationFunctionType.Exp,
                        bias=V[f"be{L}"], scale=V[f"g{L}"],
                    )
                    hT = hpool.tile([P, SN], BF16, tag="hT")
                    nc.scalar.activation(
                        hT[:], ez[:], mybir.ActivationFunctionType.Ln,
                        bias=half[:, 0:1], scale=0.5,
                    )
                    aT = hT
                nc.sync.dma_start(
                    out=out_h[:, st * SN : (st + 1) * SN], in_=aT[:]
                )

    if not nc.is_finalized():
        nc.finalize()
    return nc


def kernel(
    x, edge_index, edge_attr,
    W1, b1, g1, be1, W2, b2, g2, be2, W3, b3, g3, be3,
):
    global LAST_RESULT
    W1 = np.asarray(W1, np.float32)
    W2 = np.asarray(W2, np.float32)
    W3 = np.asarray(W3, np.float32)
    b1 = np.asarray(b1, np.float32)
    b2 = np.asarray(b2, np.float32)
    b3 = np.asarray(b3, np.float32)

    # Fold the LayerNorm mean into weights/biases: W' = W - rowmean, so the
    # matmul output is exactly mean-centered over the hidden dim.
    W1c = W1 - W1.mean(axis=1, keepdims=True)
    W2c = W2 - W2.mean(axis=1, keepdims=True)
    W3c = W3 - W3.mean(axis=1, keepdims=True)
    b1c = b1 - b1.mean()
    b2c = b2 - b2.mean()
    b3c = b3 - b3.mean()

    K, per_core = _host_prep(x, edge_index, edge_attr)
    nc = _build_program(K)

    vecs = np.stack(
        [np.asarray(v, np.float32) for v in (g1, g2, g3, be1, be2, be3)],
        axis=1,
    )  # [128, 6], column order must match VIDX in _build_program
    shared = {
        "w1a": np.ascontiguousarray(W1c[:P]).astype(ml_dtypes.bfloat16),
        "w1b": np.ascontiguousarray(W1c[P:]).astype(ml_dtypes.bfloat16),
        "w2": W2c.astype(ml_dtypes.bfloat16),
        "w3": W3c.astype(ml_dtypes.bfloat16),
        "b1": b1c.reshape(1, P).astype(ml_dtypes.bfloat16),
        "b2": b2c.reshape(1, P).astype(ml_dtypes.bfloat16),
        "b3": b3c.reshape(1, P).astype(ml_dtypes.bfloat16),
        "vecs": np.ascontiguousarray(vecs),
        "iota": np.ascontiguousarray(
            np.broadcast_to(
                np.tile(np.arange(P, dtype=np.float32), K), (P, K * P)
            )
        ),
    }
    in_maps = [
        {"edges": pay_c, "cols": col_c, "xt": xt_c, **shared}
        for (pay_c, col_c, xt_c) in per_core
    ]

    trace = bool(int(os.environ.get("KERNEL_TRACE", "0")))
    res = run_bass_kernel_spmd(nc, in_maps, core_ids=list(range(NC)), trace=trace)
    LAST_RESULT = res

    out = np.concatenate(
        [np.asarray(r["out"], dtype=np.float32).T for r in res.results], axis=0
    )
    return np.ascontiguousarray(out[:N])


# revision 16
# speedup vs baseline: 1.6175x; 1.6175x over previous
"""Trainium2 Bass kernel for nn_NodeModel (GNN message passing + 3-layer node MLP).

Strategy (node-parallel, 8 cores), v2 — transpose-free [h, node] dataflow:
  - Host: sort edges by destination node, bucket into 128-node tiles, pad each
    tile's edge list to K chunks of 128 edges. Nodes sharded contiguously
    across 8 cores (12800 padded nodes each, 25 super-tiles of 512).
  - LayerNorm mean is folded into the weights on host (W' = W - rowmean(W),
    b' = b - mean(b)) so every matmul output is already mean-centered.
  - Per 512-node super-tile on device:
      * aggregation: one-hot sel built by tensor_scalar(iota == col) on
        DVE/GPSIMD, chunk matmuls accumulate aggT[h, n] in PSUM.
      * per layer: bias seeded into PSUM by a rank-1 matmul (b'_row x ones),
        z_c = W'.T @ aT accumulated on top; var broadcast to all partitions
        via an all-ones stationary matmul of sq = z_c^2; rsig = exp(-.5*ln(.));
        zn = z_c * rsig; ssp via two ACT ops Exp(g*x+be), Ln(.5x+.5).
    Everything stays [h, node]; no PE transposes, no bn_stats.
  - ssp's -log2 appears exactly as ln(0.5 e^y + 0.5); output written bf16.
"""

import os
import sys

import numpy as np

sys.path.insert(0, "/opt/trn_rl_repo")

import bass_rust as _bass_rust
import ml_dtypes

from concourse import bacc, bass, hw_specs, mybir
from concourse import tile as tile_mod
from concourse.bass_utils import run_bass_kernel_spmd


class _Bacc(bacc.Bacc):
    """Bacc with the ACT table chooser pinned to the single function set
    that holds Ln+Exp+Copy+Identity. The default greedy chooser alternates
    between per-func sets, costing a ~1.3us ACT_TABLE_LOAD per switch."""

    def insert_act_table_loads(self):
        has_activation = any(
            isinstance(i, mybir.InstActivation)
            for b in self.main_func.blocks
            for i in b.instructions
        )
        if not has_activation:
            return
        keep = "natural_log_exp_and_others"
        tables = [
            (n, (s if n == keep else set()))
            for n, s in hw_specs.get_activation_tables(self.m.arch).items()
        ]
        _bass_rust.insert_act_table_loads(self, tables)


LOG2 = float(np.log(2.0))
N, E, H = 100000, 600000, 128
NC = 8
P = 128
SN = 512                 # nodes per super-tile
TPS = SN // P            # 128-node tiles per super-tile (4)
SPC = 25                 # super-tiles per core
TPC = SPC * TPS          # 128-node tiles per core (100)
NPC = TPC * P            # nodes per core (12800)
NPAD = NPC * NC          # padded node count (102400)
NT = NPAD // P           # total node tiles (800)

F32 = mybir.dt.float32
BF16 = mybir.dt.bfloat16

LAST_RESULT = None  # BassKernelResults of the most recent run (for profiling)


def _host_prep(x, edge_index, edge_attr):
    col = np.asarray(edge_index)[1].astype(np.int64)
    ea = np.ascontiguousarray(np.asarray(edge_attr, dtype=np.float32))
    order = np.argsort(col, kind="stable")
    col_s = col[order]
    tile_of = col_s >> 7
    counts = np.bincount(tile_of, minlength=NT)
    K = int(np.ceil(counts.max() / P))
    S = K * P
    starts = np.zeros(NT + 1, np.int64)
    starts[1:] = np.cumsum(counts)
    pos = np.arange(E) - starts[tile_of]
    slot = tile_of * S + pos
    slot_edge = np.zeros(NT * S, np.int64)
    slot_edge[slot] = order
    col_local = np.full(NT * S, 128.0, np.float32)
    col_local[slot] = (col_s & 127).astype(np.float32)
    payload = ea[slot_edge]  # [NT*S, H]

    x_pad = np.zeros((NPAD, H), np.float32)
    x_pad[:N] = np.asarray(x, dtype=np.float32)

    per_core = []
    for c in range(NC):
        r0, r1 = c * TPC * S, (c + 1) * TPC * S
        # ed rows = edge position within chunk, cols = (tile-chunk, h)
        pay_c = np.ascontiguousarray(
            payload[r0:r1]
            .reshape(TPC, K, P, H)
            .transpose(0, 2, 1, 3)
            .reshape(TPC * P, K * H)
            .astype(ml_dtypes.bfloat16)
        )
        col_c = np.ascontiguousarray(
            col_local[r0:r1].reshape(TPC, K, P).transpose(2, 0, 1).reshape(P, TPC * K)
        )
        # xT: [h, node] per core
        xt_c = np.ascontiguousarray(
            x_pad[c * NPC : (c + 1) * NPC].T.astype(ml_dtypes.bfloat16)
        )
        per_core.append((pay_c, col_c, xt_c))
    return K, per_core


def _build_program(K):
    # Bacc (not raw Bass): its compile pass splits multi-semaphore waits into
    # event-semaphore chains — walrus codegen allows only 1 wait per
    # instruction on this toolchain.
    nc = _Bacc("TRN2", target_bir_lowering=False, debug=False, num_devices=NC)

    edges_h = nc.dram_tensor("edges", [TPC * P, K * P], BF16, kind="ExternalInput")
    cols_h = nc.dram_tensor("cols", [P, TPC * K], F32, kind="ExternalInput")
    xt_h = nc.dram_tensor("xt", [P, NPC], BF16, kind="ExternalInput")
    w_h = {
        name: nc.dram_tensor(name, [P, P], BF16, kind="ExternalInput")
        for name in ("w1a", "w1b", "w2", "w3")
    }
    # bias rows b1',b2',b3' (already mean-centered) as [1,128] for rank-1 seed
    brow_h = {
        i: nc.dram_tensor(f"b{i}", [1, P], BF16, kind="ExternalInput")
        for i in (1, 2, 3)
    }
    # g1..g3, be1..be3 packed as columns of one tensor
    vecs_h = nc.dram_tensor("vecs", [P, 6], F32, kind="ExternalInput")
    # iota ramp tiled K times along free dim, for wide one-hot builds
    iota_h = nc.dram_tensor("iota", [P, K * P], F32, kind="ExternalInput")
    out_h = nc.dram_tensor("out", [P, NPC], BF16, kind="ExternalOutput")
    VIDX = {n: i for i, n in enumerate(("g1", "g2", "g3", "be1", "be2", "be3"))}

    n_st = int(os.environ.get("KERNEL_SPC", str(SPC)))

    with tile_mod.TileContext(nc) as tc:
        with (
            tc.tile_pool(name="const", bufs=1) as cpool,
            tc.tile_pool(name="edges", bufs=12) as epool,
            tc.tile_pool(name="xin", bufs=6) as xpool,
            tc.tile_pool(name="sel", bufs=12) as selpool,
            tc.tile_pool(name="aggs", bufs=3) as apool,
            tc.tile_pool(name="sq", bufs=7) as sqpool,
            tc.tile_pool(name="rs", bufs=4) as rspool,
            tc.tile_pool(name="zn", bufs=4) as znpool,
            tc.tile_pool(name="ez", bufs=4) as ezpool,
            tc.tile_pool(name="hout", bufs=7) as hpool,
            # PSUM budget (8 banks): 3 agg + 3 z + 2 var; z/var are freed
            # within an iteration by their single reader.
            tc.tile_pool(name="psagg", bufs=3, space="PSUM") as pagg,
            tc.tile_pool(name="psz", bufs=3, space="PSUM") as psz,
            tc.tile_pool(name="psvar", bufs=2, space="PSUM") as psvar,
        ):
            iota = cpool.tile_from(iota_h[:])
            cols = cpool.tile_from(cols_h[:])
            W = {k: cpool.tile_from(h[:], name=f"w_{k}") for k, h in w_h.items()}
            brow = {i: cpool.tile_from(h[:], name=f"b_{i}") for i, h in brow_h.items()}
            vecs = cpool.tile_from(vecs_h[:])
            V = {n: vecs[:, i : i + 1] for n, i in VIDX.items()}
            ones_rep = cpool.tile([P, P], BF16)
            nc.gpsimd.memset(ones_rep[:], 1.0)
            ones_row = cpool.tile([1, SN], BF16)
            nc.gpsimd.memset(ones_row[:], 1.0)
            eps = cpool.tile([P, 1], F32)
            nc.gpsimd.memset(eps[:], 1e-5)
            half = cpool.tile([P, 1], F32)
            nc.gpsimd.memset(half[:], 0.5)

            # ---- software-pipelined emission ----
            # Engines execute their instruction streams in order, so the
            # emission order below is skewed across super-tiles: every
            # iteration hands each engine work whose inputs were produced in
            # earlier iterations. Stage lags (loop index i):
            #   s0  load   (i)    : xT/ed DMA, sel one-hot builds (DVE)
            #   s1  agg    (i-1)  : 4*K chunk matmuls (PE)
            #   s2  aggS   (i-2)  : PSUM->SBUF copy (DVE)
            #   aL  layer head    : seed+z matmuls (PE), zc copy (DVE),
            #                       sq (POOL);       L1@i-3, L2@i-5, L3@i-7
            #   bL  layer tail    : var matmul (PE), ln/exp (ACT), zn (POOL),
            #                       ez/h (ACT);      L1@i-4, L2@i-6, L3@i-8
            #   s6  store  (i-9)
            S = [dict() for _ in range(n_st)]

            def s0(st):
                d = S[st]
                xT = xpool.tile([P, SN], BF16, tag="xT")
                nc.sync.dma_start(
                    out=xT[:], in_=xt_h[:, st * SN : (st + 1) * SN]
                )
                d["xT"] = xT
                d["ed"], d["sel"] = [], []
                for t4 in range(TPS):
                    t = st * TPS + t4
                    ed = epool.tile([P, K * P], BF16, tag="ed")
                    nc.sync.dma_start(
                        out=ed[:], in_=edges_h[t * P : (t + 1) * P, :]
                    )
                    # wide one-hot build: sel[e, k*128+n] = (cols[e,t*K+k]==n)
                    # is_equal only lowers on DVE (Pool rejects compare ops)
                    sel = selpool.tile([P, K * P], BF16, tag="sel")
                    nc.vector.tensor_tensor(
                        sel[:].rearrange("p (k n) -> p k n", k=K),
                        cols[:, t * K : (t + 1) * K].to_broadcast([P, K, P]),
                        iota[:].rearrange("p (k n) -> p k n", k=K),
                        op=mybir.AluOpType.is_equal,
                    )
                    d["ed"].append(ed)
                    d["sel"].append(sel)

            def s1(st):
                d = S[st]
                agg = pagg.tile([P, SN], F32, tag="agg")
                d["agg"] = agg
                for t4 in range(TPS):
                    for k in range(K):
                        nc.tensor.matmul(
                            out=d["agg"][:, t4 * P : (t4 + 1) * P],
                            lhsT=d["ed"][t4][:, k * P : (k + 1) * P],
                            rhs=d["sel"][t4][:, k * P : (k + 1) * P],
                            start=(k == 0),
                            stop=(k == K - 1),
                        )

            def s2(st):
                d = S[st]
                aggS = apool.tile([P, SN], BF16, tag="aggS")
                nc.vector.tensor_copy(aggS[:], d["agg"][:])
                d["aggS"] = aggS

            def a_stage(st, L):
                d = S[st]
                z = psz.tile([P, SN], F32, tag="z")
                nc.tensor.matmul(
                    out=z[:], lhsT=brow[L][:], rhs=ones_row[:],
                    start=True, stop=False,
                )
                if L == 1:
                    nc.tensor.matmul(
                        out=z[:], lhsT=W["w1a"][:], rhs=d["xT"][:],
                        start=False, stop=False,
                    )
                    nc.tensor.matmul(
                        out=z[:], lhsT=W["w1b"][:], rhs=d["aggS"][:],
                        start=False, stop=True,
                    )
                else:
                    nc.tensor.matmul(
                        out=z[:], lhsT=W[f"w{L}"][:], rhs=d[f"h{L - 1}"][:],
                        start=False, stop=True,
                    )
                # PSUM has one DVE read port: copy z to SBUF bf16, square the
                # copy (GPSIMD, 2-byte SBUF operands) for the var matmul.
                zc = sqpool.tile([P, SN], BF16, tag="zc")
                nc.vector.tensor_copy(zc[:], z[:])
                sq = sqpool.tile([P, SN], BF16, tag="sq")
                nc.gpsimd.tensor_tensor(
                    sq[:], zc[:], zc[:], op=mybir.AluOpType.mult
                )
                d[f"zc{L}"], d[f"sq{L}"] = zc, sq

            def b_stage(st, L):
                d = S[st]
                var = psvar.tile([P, SN], F32, tag="var")
                nc.tensor.matmul(
                    out=var[:], lhsT=ones_rep[:], rhs=d[f"sq{L}"][:],
                    start=True, stop=True,
                )
                # rsig = exp(-0.5 * ln(var/H + eps)) broadcast over h
                lnv = rspool.tile([P, SN], F32, tag="lnv")
                nc.scalar.activation(
                    lnv[:], var[:], mybir.ActivationFunctionType.Ln,
                    bias=eps[:, 0:1], scale=1.0 / H,
                )
                rsig = rspool.tile([P, SN], BF16, tag="rsig")
                nc.scalar.activation(
                    rsig[:], lnv[:], mybir.ActivationFunctionType.Exp,
                    scale=-0.5,
                )
                zn = znpool.tile([P, SN], BF16, tag="zn")
                nc.gpsimd.tensor_tensor(
                    zn[:], d[f"zc{L}"][:], rsig[:], op=mybir.AluOpType.mult
                )
                # ssp(y) = ln(0.5*exp(y) + 0.5), y = g*zn + be; includes the
                # -log2 shift. |zn| <= sqrt(127) so exp cannot overflow.
                ez = ezpool.tile([P, SN], F32, tag="ez")
                nc.scalar.activation(
                    ez[:], zn[:], mybir.ActivationFunctionType.Exp,
                    bias=V[f"be{L}"], scale=V[f"g{L}"],
                )
                hT = hpool.tile([P, SN], BF16, tag="hT")
                nc.scalar.activation(
                    hT[:], ez[:], mybir.ActivationFunctionType.Ln,
                    bias=half[:, 0:1], scale=0.5,
                )
                d[f"h{L}"] = hT

            def s6(st):
                nc.sync.dma_start(
                    out=out_h[:, st * SN : (st + 1) * SN], in_=S[st]["h3"][:]
                )

            def run(fn, j, *args):
                if 0 <= j < n_st:
                    fn(j, *args)

            for i in range(n_st + 10):
                # layer tails first: their var matmuls and ACT chains are
                # fully ready at iteration start
                run(b_stage, i - 4, 1)
                run(b_stage, i - 6, 2)
                run(b_stage, i - 8, 3)
                run(s0, i)
                run(s1, i - 1)
                run(s2, i - 2)
                run(a_stage, i - 3, 1)
                run(a_stage, i - 5, 2)
                run(a_stage, i - 7, 3)
                run(s6, i - 9)

    if not nc.is_finalized():
        nc.finalize()
    return nc


def kernel(
    x, edge_index, edge_attr,
    W1, b1, g1, be1, W2, b2, g2, be2, W3, b3, g3, be3,
):
    global LAST_RESULT
    W1 = np.asarray(W1, np.float32)
    W2 = np.asarray(W2, np.float32)
    W3 = np.asarray(W3, np.float32)
    b1 = np.asarray(b1, np.float32)
    b2 = np.asarray(b2, np.float32)
    b3 = np.asarray(b3, np.float32)

    # Fold the LayerNorm mean into weights/biases: W' = W - rowmean, so the
    # matmul output is exactly mean-centered over the hidden dim.
    W1c = W1 - W1.mean(axis=1, keepdims=True)
    W2c = W2 - W2.mean(axis=1, keepdims=True)
    W3c = W3 - W3.mean(axis=1, keepdims=True)
    b1c = b1 - b1.mean()
    b2c = b2 - b2.mean()
    b3c = b3 - b3.mean()

    K, per_core = _host_prep(x, edge_index, edge_attr)
    nc = _build_program(K)

    vecs = np.stack(
        [np.asarray(v, np.float32) for v in (g1, g2, g3, be1, be2, be3)],
        axis=1,
    )  # [128, 6], column order must match VIDX in _build_program
    shared = {
        "w1a": np.ascontiguousarray(W1c[:P]).astype(ml_dtypes.bfloat16),
        "w1b": np.ascontiguousarray(W1c[P:]).astype(ml_dtypes.bfloat16),
        "w2": W2c.astype(ml_dtypes.bfloat16),
        "w3": W3c.astype(ml_dtypes.bfloat16),
        "b1": b1c.reshape(1, P).astype(ml_dtypes.bfloat16),
        "b2": b2c.reshape(1, P).astype(ml_dtypes.bfloat16),
        "b3": b3c.reshape(1, P).astype(ml_dtypes.bfloat16),
        "vecs": np.ascontiguousarray(vecs),
        "iota": np.ascontiguousarray(
            np.broadcast_to(
                np.tile(np.arange(P, dtype=np.float32), K), (P, K * P)
            )
        ),
    }
    in_maps = [
        {"edges": pay_c, "cols": col_c, "xt": xt_c, **shared}
        for (pay_c, col_c, xt_c) in per_core
    ]

    trace = bool(int(os.environ.get("KERNEL_TRACE", "0")))
    res = run_bass_kernel_spmd(nc, in_maps, core_ids=list(range(NC)), trace=trace)
    LAST_RESULT = res

    out = np.concatenate(
        [np.asarray(r["out"], dtype=np.float32).T for r in res.results], axis=0
    )
    return np.ascontiguousarray(out[:N])


# revision 20
# speedup vs baseline: 1.6878x; 1.0434x over previous
"""Trainium2 Bass kernel for nn_NodeModel (GNN message passing + 3-layer node MLP).

Strategy (node-parallel, 8 cores), v2 — transpose-free [h, node] dataflow:
  - Host: sort edges by destination node, bucket into 128-node tiles, pad each
    tile's edge list to K chunks of 128 edges. Nodes sharded contiguously
    across 8 cores (12800 padded nodes each, 25 super-tiles of 512).
  - LayerNorm mean is folded into the weights on host (W' = W - rowmean(W),
    b' = b - mean(b)) so every matmul output is already mean-centered.
  - Per 512-node super-tile on device:
      * aggregation: one-hot sel built by tensor_scalar(iota == col) on
        DVE/GPSIMD, chunk matmuls accumulate aggT[h, n] in PSUM.
      * per layer: bias seeded into PSUM by a rank-1 matmul (b'_row x ones),
        z_c = W'.T @ aT accumulated on top; var broadcast to all partitions
        via an all-ones stationary matmul of sq = z_c^2; rsig = exp(-.5*ln(.));
        zn = z_c * rsig; ssp via two ACT ops Exp(g*x+be), Ln(.5x+.5).
    Everything stays [h, node]; no PE transposes, no bn_stats.
  - ssp's -log2 appears exactly as ln(0.5 e^y + 0.5); output written bf16.
"""

import os
import sys

import numpy as np

sys.path.insert(0, "/opt/trn_rl_repo")

import bass_rust as _bass_rust
import ml_dtypes

from concourse import bacc, bass, hw_specs, mybir
from concourse import tile as tile_mod
from concourse.bass_utils import run_bass_kernel_spmd


class _Bacc(bacc.Bacc):
    """Bacc with the ACT table chooser pinned to the single function set
    that holds Ln+Exp+Copy+Identity. The default greedy chooser alternates
    between per-func sets, costing a ~1.3us ACT_TABLE_LOAD per switch."""

    def insert_act_table_loads(self):
        has_activation = any(
            isinstance(i, mybir.InstActivation)
            for b in self.main_func.blocks
            for i in b.instructions
        )
        if not has_activation:
            return
        keep = "natural_log_exp_and_others"
        tables = [
            (n, (s if n == keep else set()))
            for n, s in hw_specs.get_activation_tables(self.m.arch).items()
        ]
        _bass_rust.insert_act_table_loads(self, tables)


LOG2 = float(np.log(2.0))
N, E, H = 100000, 600000, 128
NC = 8
P = 128
SN = 512                 # nodes per super-tile
TPS = SN // P            # 128-node tiles per super-tile (4)
SPC = 25                 # super-tiles per core
TPC = SPC * TPS          # 128-node tiles per core (100)
NPC = TPC * P            # nodes per core (12800)
NPAD = NPC * NC          # padded node count (102400)
NT = NPAD // P           # total node tiles (800)

F32 = mybir.dt.float32
BF16 = mybir.dt.bfloat16

LAST_RESULT = None  # BassKernelResults of the most recent run (for profiling)


def _host_prep(x, edge_index, edge_attr):
    col = np.asarray(edge_index)[1].astype(np.int64)
    ea = np.ascontiguousarray(np.asarray(edge_attr, dtype=np.float32))
    order = np.argsort(col, kind="stable")
    col_s = col[order]
    tile_of = col_s >> 7
    counts = np.bincount(tile_of, minlength=NT)
    K = int(np.ceil(counts.max() / P))
    S = K * P
    starts = np.zeros(NT + 1, np.int64)
    starts[1:] = np.cumsum(counts)
    pos = np.arange(E) - starts[tile_of]
    slot = tile_of * S + pos
    slot_edge = np.zeros(NT * S, np.int64)
    slot_edge[slot] = order
    col_local = np.full(NT * S, 128.0, np.float32)
    col_local[slot] = (col_s & 127).astype(np.float32)
    payload = ea[slot_edge]  # [NT*S, H]

    x_pad = np.zeros((NPAD, H), np.float32)
    x_pad[:N] = np.asarray(x, dtype=np.float32)

    per_core = []
    for c in range(NC):
        r0, r1 = c * TPC * S, (c + 1) * TPC * S
        # ed rows = edge position within chunk, cols = (tile-chunk, h)
        pay_c = np.ascontiguousarray(
            payload[r0:r1]
            .reshape(TPC, K, P, H)
            .transpose(0, 2, 1, 3)
            .reshape(TPC * P, K * H)
            .astype(ml_dtypes.bfloat16)
        )
        col_c = np.ascontiguousarray(
            col_local[r0:r1].reshape(TPC, K, P).transpose(2, 0, 1).reshape(P, TPC * K)
        )
        # xT: [h, node] per core
        xt_c = np.ascontiguousarray(
            x_pad[c * NPC : (c + 1) * NPC].T.astype(ml_dtypes.bfloat16)
        )
        per_core.append((pay_c, col_c, xt_c))
    return K, per_core


def _build_program(K):
    # Bacc (not raw Bass): its compile pass splits multi-semaphore waits into
    # event-semaphore chains — walrus codegen allows only 1 wait per
    # instruction on this toolchain.
    nc = _Bacc("TRN2", target_bir_lowering=False, debug=False, num_devices=NC)

    edges_h = nc.dram_tensor("edges", [TPC * P, K * P], BF16, kind="ExternalInput")
    cols_h = nc.dram_tensor("cols", [P, TPC * K], F32, kind="ExternalInput")
    xt_h = nc.dram_tensor("xt", [P, NPC], BF16, kind="ExternalInput")
    w_h = {
        name: nc.dram_tensor(name, [P, P], BF16, kind="ExternalInput")
        for name in ("w1a", "w1b", "w2", "w3")
    }
    # bias rows b1',b2',b3' (already mean-centered) as [1,128] for rank-1 seed
    brow_h = {
        i: nc.dram_tensor(f"b{i}", [1, P], BF16, kind="ExternalInput")
        for i in (1, 2, 3)
    }
    # g1..g3, be1..be3 packed as columns of one tensor
    vecs_h = nc.dram_tensor("vecs", [P, 6], F32, kind="ExternalInput")
    # iota ramp tiled K times along free dim, for wide one-hot builds
    iota_h = nc.dram_tensor("iota", [P, K * P], F32, kind="ExternalInput")
    out_h = nc.dram_tensor("out", [P, NPC], BF16, kind="ExternalOutput")
    VIDX = {n: i for i, n in enumerate(("g1", "g2", "g3", "be1", "be2", "be3"))}

    # groups of 2 super-tiles (1024 nodes): matmuls stay 512-wide (PSUM bank
    # limit), every elementwise/ACT op runs once per group at full width
    GW = 2 * SN
    n_st = int(os.environ.get("KERNEL_SPC", str(SPC)))
    groups = []
    off = 0
    while off < n_st * SN:
        w = min(GW, n_st * SN - off)
        groups.append((off, w))
        off += w
    n_g = len(groups)

    with tile_mod.TileContext(nc) as tc:
        with (
            tc.tile_pool(name="const", bufs=1) as cpool,
            tc.tile_pool(name="edges", bufs=20) as epool,
            tc.tile_pool(name="xin", bufs=5) as xpool,
            tc.tile_pool(name="sel", bufs=20) as selpool,
            tc.tile_pool(name="aggs", bufs=3) as apool,
            tc.tile_pool(name="sq", bufs=7) as sqpool,
            tc.tile_pool(name="rs", bufs=3) as rspool,
            tc.tile_pool(name="zn", bufs=3) as znpool,
            tc.tile_pool(name="ez", bufs=3) as ezpool,
            tc.tile_pool(name="hout", bufs=7) as hpool,
            # PSUM (8 banks), group tiles are 2 banks each: 1 agg + 2 z + 1 var
            tc.tile_pool(name="psagg", bufs=1, space="PSUM") as pagg,
            tc.tile_pool(name="psz", bufs=2, space="PSUM") as psz,
            tc.tile_pool(name="psvar", bufs=1, space="PSUM") as psvar,
        ):
            iota = cpool.tile_from(iota_h[:])
            cols = cpool.tile_from(cols_h[:])
            W = {k: cpool.tile_from(h[:], name=f"w_{k}") for k, h in w_h.items()}
            brow = {i: cpool.tile_from(h[:], name=f"b_{i}") for i, h in brow_h.items()}
            vecs = cpool.tile_from(vecs_h[:])
            V = {n: vecs[:, i : i + 1] for n, i in VIDX.items()}
            ones_rep = cpool.tile([P, P], BF16)
            nc.gpsimd.memset(ones_rep[:], 1.0)
            ones_row = cpool.tile([1, SN], BF16)
            nc.gpsimd.memset(ones_row[:], 1.0)
            eps = cpool.tile([P, 1], F32)
            nc.gpsimd.memset(eps[:], 1e-5)
            half = cpool.tile([P, 1], F32)
            nc.gpsimd.memset(half[:], 0.5)

            # ---- software-pipelined emission ----
            # Engines execute their instruction streams in order, so the
            # emission order below is skewed across super-tiles: every
            # iteration hands each engine work whose inputs were produced in
            # earlier iterations. Stage lags (loop index i):
            #   s0  load   (i)    : xT/ed DMA, sel one-hot builds (DVE)
            #   s1  agg    (i-1)  : 8*K chunk matmuls (PE)
            #   s2  aggS   (i-2)  : PSUM->SBUF copy (DVE)
            #   aL  layer head    : seed+z matmuls (PE), zc copy (DVE),
            #                       sq (POOL);       L1@i-3, L2@i-5, L3@i-7
            #   bL  layer tail    : var matmuls (PE), ln/exp (ACT), zn (POOL),
            #                       ez/h (ACT);      L1@i-4, L2@i-6, L3@i-8
            #   s6  store  (i-9)
            S = [dict() for _ in range(n_g)]

            def s0(g):
                off, w = groups[g]
                d = S[g]
                xT = xpool.tile([P, GW], BF16, tag="xT")
                nc.sync.dma_start(out=xT[:, :w], in_=xt_h[:, off : off + w])
                d["xT"] = xT
                d["ed"], d["sel"] = [], []
                for tg in range(w // P):
                    t = off // P + tg
                    ed = epool.tile([P, K * P], BF16, tag="ed")
                    nc.sync.dma_start(
                        out=ed[:], in_=edges_h[t * P : (t + 1) * P, :]
                    )
                    # wide one-hot build: sel[e, k*128+n] = (cols[e,t*K+k]==n)
                    # is_equal only lowers on DVE (Pool rejects compare ops)
                    sel = selpool.tile([P, K * P], BF16, tag="sel")
                    nc.vector.tensor_tensor(
                        sel[:].rearrange("p (k n) -> p k n", k=K),
                        cols[:, t * K : (t + 1) * K].to_broadcast([P, K, P]),
                        iota[:].rearrange("p (k n) -> p k n", k=K),
                        op=mybir.AluOpType.is_equal,
                    )
                    d["ed"].append(ed)
                    d["sel"].append(sel)

            def s1(g):
                off, w = groups[g]
                d = S[g]
                agg = pagg.tile([P, GW], F32, tag="agg")
                d["agg"] = agg
                for tg in range(w // P):
                    for k in range(K):
                        nc.tensor.matmul(
                            out=agg[:, tg * P : (tg + 1) * P],
                            lhsT=d["ed"][tg][:, k * P : (k + 1) * P],
                            rhs=d["sel"][tg][:, k * P : (k + 1) * P],
                            start=(k == 0),
                            stop=(k == K - 1),
                        )

            def s2(g):
                off, w = groups[g]
                d = S[g]
                aggS = apool.tile([P, GW], BF16, tag="aggS")
                nc.vector.tensor_copy(aggS[:, :w], d["agg"][:, :w])
                d["aggS"] = aggS

            def a_stage(g, L):
                off, w = groups[g]
                d = S[g]
                z = psz.tile([P, GW], F32, tag="z")
                for h0 in range(0, w, SN):
                    hs = slice(h0, h0 + SN)
                    nc.tensor.matmul(
                        out=z[:, hs], lhsT=brow[L][:], rhs=ones_row[:],
                        start=True, stop=False,
                    )
                    if L == 1:
                        nc.tensor.matmul(
                            out=z[:, hs], lhsT=W["w1a"][:], rhs=d["xT"][:, hs],
                            start=False, stop=False,
                        )
                        nc.tensor.matmul(
                            out=z[:, hs], lhsT=W["w1b"][:],
                            rhs=d["aggS"][:, hs],
                            start=False, stop=True,
                        )
                    else:
                        nc.tensor.matmul(
                            out=z[:, hs], lhsT=W[f"w{L}"][:],
                            rhs=d[f"h{L - 1}"][:, hs],
                            start=False, stop=True,
                        )
                # PSUM has one DVE read port: copy z to SBUF bf16, square the
                # copy (GPSIMD, 2-byte SBUF operands) for the var matmuls.
                zc = sqpool.tile([P, GW], BF16, tag="zc")
                nc.vector.tensor_copy(zc[:, :w], z[:, :w])
                sq = sqpool.tile([P, GW], BF16, tag="sq")
                nc.gpsimd.tensor_tensor(
                    sq[:, :w], zc[:, :w], zc[:, :w], op=mybir.AluOpType.mult
                )
                d[f"zc{L}"], d[f"sq{L}"] = zc, sq

            def b_stage(g, L):
                off, w = groups[g]
                d = S[g]
                var = psvar.tile([P, GW], F32, tag="var")
                for h0 in range(0, w, SN):
                    hs = slice(h0, h0 + SN)
                    nc.tensor.matmul(
                        out=var[:, hs], lhsT=ones_rep[:],
                        rhs=d[f"sq{L}"][:, hs],
                        start=True, stop=True,
                    )
                # rsig = exp(-0.5 * ln(var/H + eps)) broadcast over h
                lnv = rspool.tile([P, GW], BF16, tag="lnv")
                nc.scalar.activation(
                    lnv[:, :w], var[:, :w], mybir.ActivationFunctionType.Ln,
                    bias=eps[:, 0:1], scale=1.0 / H,
                )
                rsig = rspool.tile([P, GW], BF16, tag="rsig")
                nc.scalar.activation(
                    rsig[:, :w], lnv[:, :w], mybir.ActivationFunctionType.Exp,
                    scale=-0.5,
                )
                zn = znpool.tile([P, GW], BF16, tag="zn")
                nc.gpsimd.tensor_tensor(
                    zn[:, :w], d[f"zc{L}"][:, :w], rsig[:, :w],
                    op=mybir.AluOpType.mult,
                )
                # ssp(y) = ln(0.5*exp(y) + 0.5), y = g*zn + be; includes the
                # -log2 shift. |zn| <= sqrt(127) so exp cannot overflow.
                ez = ezpool.tile([P, GW], BF16, tag="ez")
                nc.scalar.activation(
                    ez[:, :w], zn[:, :w], mybir.ActivationFunctionType.Exp,
                    bias=V[f"be{L}"], scale=V[f"g{L}"],
                )
                hT = hpool.tile([P, GW], BF16, tag="hT")
                nc.scalar.activation(
                    hT[:, :w], ez[:, :w], mybir.ActivationFunctionType.Ln,
                    bias=half[:, 0:1], scale=0.5,
                )
                d[f"h{L}"] = hT

            def s6(g):
                off, w = groups[g]
                nc.sync.dma_start(
                    out=out_h[:, off : off + w], in_=S[g]["h3"][:, :w]
                )

            def run(fn, j, *args):
                if 0 <= j < n_g:
                    fn(j, *args)

            for i in range(n_g + 10):
                # aggS copy first (frees the agg PSUM bank), then layer
                # tails: their var matmuls and ACT chains are ready at
                # iteration start
                run(s2, i - 2)
                run(b_stage, i - 4, 1)
                run(b_stage, i - 6, 2)
                run(s0, i)
                run(s1, i - 1)
                run(b_stage, i - 8, 3)
                run(a_stage, i - 3, 1)
                run(a_stage, i - 5, 2)
                run(a_stage, i - 7, 3)
                run(s6, i - 9)

    if not nc.is_finalized():
        nc.finalize()
    return nc


def kernel(
    x, edge_index, edge_attr,
    W1, b1, g1, be1, W2, b2, g2, be2, W3, b3, g3, be3,
):
    global LAST_RESULT
    W1 = np.asarray(W1, np.float32)
    W2 = np.asarray(W2, np.float32)
    W3 = np.asarray(W3, np.float32)
    b1 = np.asarray(b1, np.float32)
    b2 = np.asarray(b2, np.float32)
    b3 = np.asarray(b3, np.float32)

    # Fold the LayerNorm mean into weights/biases: W' = W - rowmean, so the
    # matmul output is exactly mean-centered over the hidden dim.
    W1c = W1 - W1.mean(axis=1, keepdims=True)
    W2c = W2 - W2.mean(axis=1, keepdims=True)
    W3c = W3 - W3.mean(axis=1, keepdims=True)
    b1c = b1 - b1.mean()
    b2c = b2 - b2.mean()
    b3c = b3 - b3.mean()

    K, per_core = _host_prep(x, edge_index, edge_attr)
    nc = _build_program(K)

    vecs = np.stack(
        [np.asarray(v, np.float32) for v in (g1, g2, g3, be1, be2, be3)],
        axis=1,
    )  # [128, 6], column order must match VIDX in _build_program
    shared = {
        "w1a": np.ascontiguousarray(W1c[:P]).astype(ml_dtypes.bfloat16),
        "w1b": np.ascontiguousarray(W1c[P:]).astype(ml_dtypes.bfloat16),
        "w2": W2c.astype(ml_dtypes.bfloat16),
        "w3": W3c.astype(ml_dtypes.bfloat16),
        "b1": b1c.reshape(1, P).astype(ml_dtypes.bfloat16),
        "b2": b2c.reshape(1, P).astype(ml_dtypes.bfloat16),
        "b3": b3c.reshape(1, P).astype(ml_dtypes.bfloat16),
        "vecs": np.ascontiguousarray(vecs),
        "iota": np.ascontiguousarray(
            np.broadcast_to(
                np.tile(np.arange(P, dtype=np.float32), K), (P, K * P)
            )
        ),
    }
    in_maps = [
        {"edges": pay_c, "cols": col_c, "xt": xt_c, **shared}
        for (pay_c, col_c, xt_c) in per_core
    ]

    trace = bool(int(os.environ.get("KERNEL_TRACE", "0")))
    res = run_bass_kernel_spmd(nc, in_maps, core_ids=list(range(NC)), trace=trace)
    LAST_RESULT = res

    out = np.concatenate(
        [np.asarray(r["out"], dtype=np.float32).T for r in res.results], axis=0
    )
    return np.ascontiguousarray(out[:N])


# revision 25
# speedup vs baseline: 1.8937x; 1.1220x over previous
"""Trainium2 Bass kernel for nn_NodeModel (GNN message passing + 3-layer node MLP).

Strategy (node-parallel, 8 cores), v2 — transpose-free [h, node] dataflow:
  - Host: sort edges by destination node, bucket into 128-node tiles, pad each
    tile's edge list to K chunks of 128 edges. Nodes sharded contiguously
    across 8 cores (12800 padded nodes each, 25 super-tiles of 512).
  - LayerNorm mean is folded into the weights on host (W' = W - rowmean(W),
    b' = b - mean(b)) so every matmul output is already mean-centered.
  - Per 512-node super-tile on device:
      * aggregation: one-hot sel built by tensor_scalar(iota == col) on
        DVE/GPSIMD, chunk matmuls accumulate aggT[h, n] in PSUM.
      * per layer: bias seeded into PSUM by a rank-1 matmul (b'_row x ones),
        z_c = W'.T @ aT accumulated on top; var broadcast to all partitions
        via an all-ones stationary matmul of sq = z_c^2; rsig = exp(-.5*ln(.));
        zn = z_c * rsig; ssp via two ACT ops Exp(g*x+be), Ln(.5x+.5).
    Everything stays [h, node]; no PE transposes, no bn_stats.
  - ssp's -log2 appears exactly as ln(0.5 e^y + 0.5); output written bf16.
"""

import os
import sys

import numpy as np

sys.path.insert(0, "/opt/trn_rl_repo")

import bass_rust as _bass_rust
import ml_dtypes

from concourse import bacc, bass, hw_specs, mybir
from concourse import tile as tile_mod
from concourse.bass_utils import run_bass_kernel_spmd


class _Bacc(bacc.Bacc):
    """Bacc with the ACT table chooser pinned to the single function set
    that holds Ln+Exp+Copy+Identity. The default greedy chooser alternates
    between per-func sets, costing a ~1.3us ACT_TABLE_LOAD per switch."""

    def insert_act_table_loads(self):
        has_activation = any(
            isinstance(i, mybir.InstActivation)
            for b in self.main_func.blocks
            for i in b.instructions
        )
        if not has_activation:
            return
        keep = "natural_log_exp_and_others"
        tables = [
            (n, (s if n == keep else set()))
            for n, s in hw_specs.get_activation_tables(self.m.arch).items()
        ]
        _bass_rust.insert_act_table_loads(self, tables)


LOG2 = float(np.log(2.0))
N, E, H = 100000, 600000, 128
NC = 8
P = 128
SN = 512                 # nodes per super-tile
TPS = SN // P            # 128-node tiles per super-tile (4)
SPC = 25                 # super-tiles per core
TPC = SPC * TPS          # 128-node tiles per core (100)
NPC = TPC * P            # nodes per core (12800)
NPAD = NPC * NC          # padded node count (102400)
NT = NPAD // P           # total node tiles (800)

F32 = mybir.dt.float32
BF16 = mybir.dt.bfloat16

LAST_RESULT = None  # BassKernelResults of the most recent run (for profiling)


def _host_prep(x, edge_index, edge_attr):
    col = np.asarray(edge_index)[1].astype(np.int64)
    ea = np.ascontiguousarray(np.asarray(edge_attr, dtype=np.float32))
    order = np.argsort(col, kind="stable")
    col_s = col[order]
    tile_of = col_s >> 7
    counts = np.bincount(tile_of, minlength=NT)
    K = int(np.ceil(counts.max() / P))
    S = K * P
    starts = np.zeros(NT + 1, np.int64)
    starts[1:] = np.cumsum(counts)
    pos = np.arange(E) - starts[tile_of]
    slot = tile_of * S + pos
    slot_edge = np.zeros(NT * S, np.int64)
    slot_edge[slot] = order
    col_local = np.full(NT * S, 128.0, np.float32)
    col_local[slot] = (col_s & 127).astype(np.float32)
    payload = ea[slot_edge]  # [NT*S, H]

    # host-built one-hot sel, same row layout as the edge payload:
    # row (tile, edge-pos-in-chunk), col (chunk, local node)
    sel_full = np.zeros((NT * P, K * P), ml_dtypes.bfloat16)
    k_arr = (pos // P).astype(np.int64)
    e_arr = (pos % P).astype(np.int64)
    sel_full[tile_of * P + e_arr, k_arr * P + (col_s & 127)] = 1.0

    x_pad = np.zeros((NPAD, H), np.float32)
    x_pad[:N] = np.asarray(x, dtype=np.float32)

    per_core = []
    for c in range(NC):
        r0, r1 = c * TPC * S, (c + 1) * TPC * S
        # ed rows = edge position within chunk, cols = (tile-chunk, h)
        pay_c = np.ascontiguousarray(
            payload[r0:r1]
            .reshape(TPC, K, P, H)
            .transpose(0, 2, 1, 3)
            .reshape(TPC * P, K * H)
            .astype(ml_dtypes.bfloat16)
        )
        col_c = np.ascontiguousarray(
            col_local[r0:r1].reshape(TPC, K, P).transpose(2, 0, 1).reshape(P, TPC * K)
        )
        sel_c = np.ascontiguousarray(sel_full[c * TPC * P : (c + 1) * TPC * P])
        # xT: [h, node] per core
        xt_c = np.ascontiguousarray(
            x_pad[c * NPC : (c + 1) * NPC].T.astype(ml_dtypes.bfloat16)
        )
        per_core.append((pay_c, col_c, sel_c, xt_c))
    return K, per_core


def _build_program(K):
    # Bacc (not raw Bass): its compile pass splits multi-semaphore waits into
    # event-semaphore chains — walrus codegen allows only 1 wait per
    # instruction on this toolchain.
    nc = _Bacc("TRN2", target_bir_lowering=False, debug=False, num_devices=NC)

    edges_h = nc.dram_tensor("edges", [TPC * P, K * P], BF16, kind="ExternalInput")
    selh_h = nc.dram_tensor("selh", [TPC * P, K * P], BF16, kind="ExternalInput")
    cols_h = nc.dram_tensor("cols", [P, TPC * K], F32, kind="ExternalInput")
    xt_h = nc.dram_tensor("xt", [P, NPC], BF16, kind="ExternalInput")
    w_h = {
        name: nc.dram_tensor(name, [P, P], BF16, kind="ExternalInput")
        for name in ("w1a", "w1b", "w2", "w3")
    }
    # bias rows b1',b2',b3' (already mean-centered) as [1,128] for rank-1 seed
    brow_h = {
        i: nc.dram_tensor(f"b{i}", [1, P], BF16, kind="ExternalInput")
        for i in (1, 2, 3)
    }
    # g1..g3, be1..be3 packed as columns of one tensor
    vecs_h = nc.dram_tensor("vecs", [P, 6], F32, kind="ExternalInput")
    # iota ramp tiled K times along free dim, for wide one-hot builds
    iota_h = nc.dram_tensor("iota", [P, K * P], F32, kind="ExternalInput")
    out_h = nc.dram_tensor("out", [P, NPC], BF16, kind="ExternalOutput")
    VIDX = {n: i for i, n in enumerate(("g1", "g2", "g3", "be1", "be2", "be3"))}

    # groups of 2 super-tiles (1024 nodes): matmuls stay 512-wide (PSUM bank
    # limit), every elementwise/ACT op runs once per group at full width
    GW = 2 * SN
    sel_ship = int(os.environ.get("KERNEL_SELH", "8"))  # of 8 tiles per group
    n_st = int(os.environ.get("KERNEL_SPC", str(SPC)))
    groups = []
    off = 0
    while off < n_st * SN:
        w = min(GW, n_st * SN - off)
        groups.append((off, w))
        off += w
    n_g = len(groups)

    with tile_mod.TileContext(nc) as tc:
        with (
            tc.tile_pool(name="const", bufs=1) as cpool,
            tc.tile_pool(name="edges", bufs=20) as epool,
            tc.tile_pool(name="xin", bufs=5) as xpool,
            tc.tile_pool(name="sel", bufs=20) as selpool,
            tc.tile_pool(name="aggs", bufs=3) as apool,
            tc.tile_pool(name="sq", bufs=7) as sqpool,
            tc.tile_pool(name="rs", bufs=3) as rspool,
            tc.tile_pool(name="zn", bufs=3) as znpool,
            tc.tile_pool(name="ez", bufs=3) as ezpool,
            tc.tile_pool(name="hout", bufs=7) as hpool,
            # PSUM (8 banks), group tiles are 2 banks each: 1 agg + 2 z + 1 var
            tc.tile_pool(name="psagg", bufs=1, space="PSUM") as pagg,
            tc.tile_pool(name="psz", bufs=2, space="PSUM") as psz,
            tc.tile_pool(name="psvar", bufs=1, space="PSUM") as psvar,
        ):
            iota = cpool.tile_from(iota_h[:])
            cols = cpool.tile_from(cols_h[:])
            W = {k: cpool.tile_from(h[:], name=f"w_{k}") for k, h in w_h.items()}
            brow = {i: cpool.tile_from(h[:], name=f"b_{i}") for i, h in brow_h.items()}
            vecs = cpool.tile_from(vecs_h[:])
            V = {n: vecs[:, i : i + 1] for n, i in VIDX.items()}
            ones_rep = cpool.tile([P, P], BF16)
            nc.gpsimd.memset(ones_rep[:], 1.0)
            ones_row = cpool.tile([1, SN], BF16)
            nc.gpsimd.memset(ones_row[:], 1.0)
            eps = cpool.tile([P, 1], F32)
            nc.gpsimd.memset(eps[:], 1e-5)
            half = cpool.tile([P, 1], F32)
            nc.gpsimd.memset(half[:], 0.5)

            # ---- software-pipelined emission ----
            # Engines execute their instruction streams in order, so the
            # emission order below is skewed across super-tiles: every
            # iteration hands each engine work whose inputs were produced in
            # earlier iterations. Stage lags (loop index i):
            #   s0  load   (i)    : xT/ed DMA, sel one-hot builds (DVE)
            #   s1  agg    (i-1)  : 8*K chunk matmuls (PE)
            #   s2  aggS   (i-2)  : PSUM->SBUF copy (DVE)
            #   aL  layer head    : seed+z matmuls (PE), zc copy (DVE),
            #                       sq (POOL);       L1@i-3, L2@i-5, L3@i-7
            #   bL  layer tail    : var matmuls (PE), ln/exp (ACT), zn (POOL),
            #                       ez/h (ACT);      L1@i-4, L2@i-6, L3@i-8
            #   s6  store  (i-9)
            S = [dict() for _ in range(n_g)]

            def s0(g):
                off, w = groups[g]
                d = S[g]
                xT = xpool.tile([P, GW], BF16, tag="xT")
                nc.sync.dma_start(out=xT[:, :w], in_=xt_h[:, off : off + w])
                d["xT"] = xT
                d["ed"], d["sel"] = [], []
                for tg in range(w // P):
                    t = off // P + tg
                    ed = epool.tile([P, K * P], BF16, tag="ed")
                    nc.sync.dma_start(
                        out=ed[:], in_=edges_h[t * P : (t + 1) * P, :]
                    )
                    sel = selpool.tile([P, K * P], BF16, tag="sel")
                    if tg % 8 < sel_ship:
                        # host-prebuilt one-hot, DMA'd like the payload
                        nc.sync.dma_start(
                            out=sel[:], in_=selh_h[t * P : (t + 1) * P, :]
                        )
                    else:
                        # on-device build: sel[e, k*128+n] = (cols[e,t*K+k]==n)
                        # is_equal only lowers on DVE (Pool rejects compares)
                        nc.vector.tensor_tensor(
                            sel[:].rearrange("p (k n) -> p k n", k=K),
                            cols[:, t * K : (t + 1) * K].to_broadcast(
                                [P, K, P]
                            ),
                            iota[:].rearrange("p (k n) -> p k n", k=K),
                            op=mybir.AluOpType.is_equal,
                        )
                    d["ed"].append(ed)
                    d["sel"].append(sel)

            def s1(g):
                off, w = groups[g]
                d = S[g]
                agg = pagg.tile([P, GW], F32, tag="agg")
                d["agg"] = agg
                for tg in range(w // P):
                    for k in range(K):
                        nc.tensor.matmul(
                            out=agg[:, tg * P : (tg + 1) * P],
                            lhsT=d["ed"][tg][:, k * P : (k + 1) * P],
                            rhs=d["sel"][tg][:, k * P : (k + 1) * P],
                            start=(k == 0),
                            stop=(k == K - 1),
                        )

            def s2(g):
                off, w = groups[g]
                d = S[g]
                aggS = apool.tile([P, GW], BF16, tag="aggS")
                nc.vector.tensor_copy(aggS[:, :w], d["agg"][:, :w])
                d["aggS"] = aggS

            def a_stage(g, L):
                off, w = groups[g]
                d = S[g]
                z = psz.tile([P, GW], F32, tag="z")
                for h0 in range(0, w, SN):
                    hs = slice(h0, h0 + SN)
                    nc.tensor.matmul(
                        out=z[:, hs], lhsT=brow[L][:], rhs=ones_row[:],
                        start=True, stop=False,
                    )
                    if L == 1:
                        nc.tensor.matmul(
                            out=z[:, hs], lhsT=W["w1a"][:], rhs=d["xT"][:, hs],
                            start=False, stop=False,
                        )
                        nc.tensor.matmul(
                            out=z[:, hs], lhsT=W["w1b"][:],
                            rhs=d["aggS"][:, hs],
                            start=False, stop=True,
                        )
                    else:
                        nc.tensor.matmul(
                            out=z[:, hs], lhsT=W[f"w{L}"][:],
                            rhs=d[f"h{L - 1}"][:, hs],
                            start=False, stop=True,
                        )
                # PSUM has one DVE read port: copy z to SBUF bf16, square the
                # copy (GPSIMD, 2-byte SBUF operands) for the var matmuls.
                zc = sqpool.tile([P, GW], BF16, tag="zc")
                nc.vector.tensor_copy(zc[:, :w], z[:, :w])
                sq = sqpool.tile([P, GW], BF16, tag="sq")
                nc.gpsimd.tensor_tensor(
                    sq[:, :w], zc[:, :w], zc[:, :w], op=mybir.AluOpType.mult
                )
                d[f"zc{L}"], d[f"sq{L}"] = zc, sq

            def b_stage(g, L):
                off, w = groups[g]
                d = S[g]
                var = psvar.tile([P, GW], F32, tag="var")
                for h0 in range(0, w, SN):
                    hs = slice(h0, h0 + SN)
                    nc.tensor.matmul(
                        out=var[:, hs], lhsT=ones_rep[:],
                        rhs=d[f"sq{L}"][:, hs],
                        start=True, stop=True,
                    )
                # rsig = exp(-0.5 * ln(var/H + eps)) broadcast over h
                lnv = rspool.tile([P, GW], BF16, tag="lnv")
                nc.scalar.activation(
                    lnv[:, :w], var[:, :w], mybir.ActivationFunctionType.Ln,
                    bias=eps[:, 0:1], scale=1.0 / H,
                )
                rsig = rspool.tile([P, GW], BF16, tag="rsig")
                nc.scalar.activation(
                    rsig[:, :w], lnv[:, :w], mybir.ActivationFunctionType.Exp,
                    scale=-0.5,
                )
                zn = znpool.tile([P, GW], BF16, tag="zn")
                nc.gpsimd.tensor_tensor(
                    zn[:, :w], d[f"zc{L}"][:, :w], rsig[:, :w],
                    op=mybir.AluOpType.mult,
                )
                # ssp(y) = ln(0.5*exp(y) + 0.5), y = g*zn + be; includes the
                # -log2 shift. |zn| <= sqrt(127) so exp cannot overflow.
                ez = ezpool.tile([P, GW], BF16, tag="ez")
                nc.scalar.activation(
                    ez[:, :w], zn[:, :w], mybir.ActivationFunctionType.Exp,
                    bias=V[f"be{L}"], scale=V[f"g{L}"],
                )
                hT = hpool.tile([P, GW], BF16, tag="hT")
                nc.scalar.activation(
                    hT[:, :w], ez[:, :w], mybir.ActivationFunctionType.Ln,
                    bias=half[:, 0:1], scale=0.5,
                )
                d[f"h{L}"] = hT

            def s6(g):
                off, w = groups[g]
                nc.sync.dma_start(
                    out=out_h[:, off : off + w], in_=S[g]["h3"][:, :w]
                )

            def run(fn, j, *args):
                if 0 <= j < n_g:
                    fn(j, *args)

            for i in range(n_g + 10):
                # aggS copy first (frees the agg PSUM bank), then layer
                # tails: their var matmuls and ACT chains are ready at
                # iteration start
                run(s2, i - 2)
                run(b_stage, i - 4, 1)
                run(b_stage, i - 6, 2)
                run(s0, i)
                run(s1, i - 1)
                run(b_stage, i - 8, 3)
                run(a_stage, i - 3, 1)
                run(a_stage, i - 5, 2)
                run(a_stage, i - 7, 3)
                run(s6, i - 9)

    if not nc.is_finalized():
        nc.finalize()
    return nc


def kernel(
    x, edge_index, edge_attr,
    W1, b1, g1, be1, W2, b2, g2, be2, W3, b3, g3, be3,
):
    global LAST_RESULT
    W1 = np.asarray(W1, np.float32)
    W2 = np.asarray(W2, np.float32)
    W3 = np.asarray(W3, np.float32)
    b1 = np.asarray(b1, np.float32)
    b2 = np.asarray(b2, np.float32)
    b3 = np.asarray(b3, np.float32)

    # Fold the LayerNorm mean into weights/biases: W' = W - rowmean, so the
    # matmul output is exactly mean-centered over the hidden dim.
    W1c = W1 - W1.mean(axis=1, keepdims=True)
    W2c = W2 - W2.mean(axis=1, keepdims=True)
    W3c = W3 - W3.mean(axis=1, keepdims=True)
    b1c = b1 - b1.mean()
    b2c = b2 - b2.mean()
    b3c = b3 - b3.mean()

    K, per_core = _host_prep(x, edge_index, edge_attr)
    nc = _build_program(K)

    vecs = np.stack(
        [np.asarray(v, np.float32) for v in (g1, g2, g3, be1, be2, be3)],
        axis=1,
    )  # [128, 6], column order must match VIDX in _build_program
    shared = {
        "w1a": np.ascontiguousarray(W1c[:P]).astype(ml_dtypes.bfloat16),
        "w1b": np.ascontiguousarray(W1c[P:]).astype(ml_dtypes.bfloat16),
        "w2": W2c.astype(ml_dtypes.bfloat16),
        "w3": W3c.astype(ml_dtypes.bfloat16),
        "b1": b1c.reshape(1, P).astype(ml_dtypes.bfloat16),
        "b2": b2c.reshape(1, P).astype(ml_dtypes.bfloat16),
        "b3": b3c.reshape(1, P).astype(ml_dtypes.bfloat16),
        "vecs": np.ascontiguousarray(vecs),
        "iota": np.ascontiguousarray(
            np.broadcast_to(
                np.tile(np.arange(P, dtype=np.float32), K), (P, K * P)
            )
        ),
    }
    in_maps = [
        {"edges": pay_c, "cols": col_c, "selh": sel_c, "xt": xt_c, **shared}
        for (pay_c, col_c, sel_c, xt_c) in per_core
    ]

    trace = bool(int(os.environ.get("KERNEL_TRACE", "0")))
    res = run_bass_kernel_spmd(nc, in_maps, core_ids=list(range(NC)), trace=trace)
    LAST_RESULT = res

    out = np.concatenate(
        [np.asarray(r["out"], dtype=np.float32).T for r in res.results], axis=0
    )
    return np.ascontiguousarray(out[:N])


# revision 28
# speedup vs baseline: 1.9046x; 1.0058x over previous
"""Trainium2 Bass kernel for nn_NodeModel (GNN message passing + 3-layer node MLP).

Strategy (node-parallel, 8 cores), v2 — transpose-free [h, node] dataflow:
  - Host: sort edges by destination node, bucket into 128-node tiles, pad each
    tile's edge list to K chunks of 128 edges. Nodes sharded contiguously
    across 8 cores (12800 padded nodes each, 25 super-tiles of 512).
  - LayerNorm mean is folded into the weights on host (W' = W - rowmean(W),
    b' = b - mean(b)) so every matmul output is already mean-centered.
  - Per 512-node super-tile on device:
      * aggregation: one-hot sel built by tensor_scalar(iota == col) on
        DVE/GPSIMD, chunk matmuls accumulate aggT[h, n] in PSUM.
      * per layer: bias seeded into PSUM by a rank-1 matmul (b'_row x ones),
        z_c = W'.T @ aT accumulated on top; var broadcast to all partitions
        via an all-ones stationary matmul of sq = z_c^2; rsig = exp(-.5*ln(.));
        zn = z_c * rsig; ssp via two ACT ops Exp(g*x+be), Ln(.5x+.5).
    Everything stays [h, node]; no PE transposes, no bn_stats.
  - ssp's -log2 appears exactly as ln(0.5 e^y + 0.5); output written bf16.
"""

import os
import sys

import numpy as np

sys.path.insert(0, "/opt/trn_rl_repo")

import bass_rust as _bass_rust
import ml_dtypes

from concourse import bacc, bass, hw_specs, mybir
from concourse import tile as tile_mod
from concourse.bass_utils import run_bass_kernel_spmd


class _Bacc(bacc.Bacc):
    """Bacc with the ACT table chooser pinned to the single function set
    that holds Ln+Exp+Copy+Identity. The default greedy chooser alternates
    between per-func sets, costing a ~1.3us ACT_TABLE_LOAD per switch."""

    def insert_act_table_loads(self):
        has_activation = any(
            isinstance(i, mybir.InstActivation)
            for b in self.main_func.blocks
            for i in b.instructions
        )
        if not has_activation:
            return
        keep = "natural_log_exp_and_others"
        tables = [
            (n, (s if n == keep else set()))
            for n, s in hw_specs.get_activation_tables(self.m.arch).items()
        ]
        _bass_rust.insert_act_table_loads(self, tables)


LOG2 = float(np.log(2.0))
N, E, H = 100000, 600000, 128
NC = 8
P = 128
SN = 512                 # nodes per super-tile
TPS = SN // P            # 128-node tiles per super-tile (4)
SPC = 25                 # super-tiles per core
TPC = SPC * TPS          # 128-node tiles per core (100)
NPC = TPC * P            # nodes per core (12800)
NPAD = NPC * NC          # padded node count (102400)
NT = NPAD // P           # total node tiles (800)

F32 = mybir.dt.float32
BF16 = mybir.dt.bfloat16

LAST_RESULT = None  # BassKernelResults of the most recent run (for profiling)


def _host_prep(x, edge_index, edge_attr):
    col = np.asarray(edge_index)[1].astype(np.int64)
    ea = np.ascontiguousarray(np.asarray(edge_attr, dtype=np.float32))
    order = np.argsort(col, kind="stable")
    col_s = col[order]
    tile_of = col_s >> 7
    counts = np.bincount(tile_of, minlength=NT)
    K = int(np.ceil(counts.max() / P))
    S = K * P
    starts = np.zeros(NT + 1, np.int64)
    starts[1:] = np.cumsum(counts)
    pos = np.arange(E) - starts[tile_of]
    slot = tile_of * S + pos
    slot_edge = np.zeros(NT * S, np.int64)
    slot_edge[slot] = order
    col_local = np.full(NT * S, 128.0, np.float32)
    col_local[slot] = (col_s & 127).astype(np.float32)
    payload = ea[slot_edge]  # [NT*S, H]

    # host-built one-hot sel, same row layout as the edge payload:
    # row (tile, edge-pos-in-chunk), col (chunk, local node)
    sel_full = np.zeros((NT * P, K * P), ml_dtypes.bfloat16)
    k_arr = (pos // P).astype(np.int64)
    e_arr = (pos % P).astype(np.int64)
    sel_full[tile_of * P + e_arr, k_arr * P + (col_s & 127)] = 1.0

    x_pad = np.zeros((NPAD, H), np.float32)
    x_pad[:N] = np.asarray(x, dtype=np.float32)

    per_core = []
    for c in range(NC):
        r0, r1 = c * TPC * S, (c + 1) * TPC * S
        # ed rows = edge position within chunk, cols = (tile-chunk, h)
        pay_c = np.ascontiguousarray(
            payload[r0:r1]
            .reshape(TPC, K, P, H)
            .transpose(0, 2, 1, 3)
            .reshape(TPC * P, K * H)
            .astype(ml_dtypes.bfloat16)
        )
        col_c = np.ascontiguousarray(
            col_local[r0:r1].reshape(TPC, K, P).transpose(2, 0, 1).reshape(P, TPC * K)
        )
        sel_c = np.ascontiguousarray(sel_full[c * TPC * P : (c + 1) * TPC * P])
        # xT: [h, node] per core
        xt_c = np.ascontiguousarray(
            x_pad[c * NPC : (c + 1) * NPC].T.astype(ml_dtypes.bfloat16)
        )
        per_core.append((pay_c, col_c, sel_c, xt_c))
    return K, per_core


def _build_program(K):
    # Bacc (not raw Bass): its compile pass splits multi-semaphore waits into
    # event-semaphore chains — walrus codegen allows only 1 wait per
    # instruction on this toolchain.
    nc = _Bacc("TRN2", target_bir_lowering=False, debug=False, num_devices=NC)

    edges_h = nc.dram_tensor("edges", [TPC * P, K * P], BF16, kind="ExternalInput")
    selh_h = nc.dram_tensor("selh", [TPC * P, K * P], BF16, kind="ExternalInput")
    cols_h = nc.dram_tensor("cols", [P, TPC * K], F32, kind="ExternalInput")
    xt_h = nc.dram_tensor("xt", [P, NPC], BF16, kind="ExternalInput")
    w_h = {
        name: nc.dram_tensor(name, [P, P], BF16, kind="ExternalInput")
        for name in ("w1a", "w1b", "w2", "w3")
    }
    # bias rows b1',b2',b3' (already mean-centered) as [1,128] for rank-1 seed
    brow_h = {
        i: nc.dram_tensor(f"b{i}", [1, P], BF16, kind="ExternalInput")
        for i in (1, 2, 3)
    }
    # g1..g3, be1..be3 packed as columns of one tensor
    vecs_h = nc.dram_tensor("vecs", [P, 6], F32, kind="ExternalInput")
    # iota ramp tiled K times along free dim, for wide one-hot builds
    iota_h = nc.dram_tensor("iota", [P, K * P], F32, kind="ExternalInput")
    out_h = nc.dram_tensor("out", [P, NPC], BF16, kind="ExternalOutput")
    VIDX = {n: i for i, n in enumerate(("g1", "g2", "g3", "be1", "be2", "be3"))}

    # groups of 2 super-tiles (1024 nodes): matmuls stay 512-wide (PSUM bank
    # limit), every elementwise/ACT op runs once per group at full width
    GW = 2 * SN
    sel_ship = int(os.environ.get("KERNEL_SELH", "8"))  # of 8 tiles per group
    n_st = int(os.environ.get("KERNEL_SPC", str(SPC)))
    groups = []
    off = 0
    while off < n_st * SN:
        w = min(GW, n_st * SN - off)
        groups.append((off, w))
        off += w
    n_g = len(groups)

    with tile_mod.TileContext(nc) as tc:
        with (
            tc.tile_pool(name="const", bufs=1) as cpool,
            tc.tile_pool(name="edges", bufs=3) as epool,
            tc.tile_pool(name="xin", bufs=5) as xpool,
            tc.tile_pool(name="sel", bufs=3) as selpool,
            tc.tile_pool(name="aggs", bufs=3) as apool,
            tc.tile_pool(name="sq", bufs=7) as sqpool,
            tc.tile_pool(name="rs", bufs=3) as rspool,
            tc.tile_pool(name="zn", bufs=3) as znpool,
            tc.tile_pool(name="ez", bufs=3) as ezpool,
            tc.tile_pool(name="hout", bufs=7) as hpool,
            # PSUM (8 banks), group tiles are 2 banks each: 1 agg + 2 z + 1 var
            tc.tile_pool(name="psagg", bufs=1, space="PSUM") as pagg,
            tc.tile_pool(name="psz", bufs=2, space="PSUM") as psz,
            tc.tile_pool(name="psvar", bufs=1, space="PSUM") as psvar,
        ):
            iota = cpool.tile_from(iota_h[:])
            cols = cpool.tile_from(cols_h[:])
            W = {k: cpool.tile_from(h[:], name=f"w_{k}") for k, h in w_h.items()}
            brow = {i: cpool.tile_from(h[:], name=f"b_{i}") for i, h in brow_h.items()}
            vecs = cpool.tile_from(vecs_h[:])
            V = {n: vecs[:, i : i + 1] for n, i in VIDX.items()}
            ones_rep = cpool.tile([P, P], BF16)
            nc.gpsimd.memset(ones_rep[:], 1.0)
            ones_row = cpool.tile([1, SN], BF16)
            nc.gpsimd.memset(ones_row[:], 1.0)
            eps = cpool.tile([P, 1], F32)
            nc.gpsimd.memset(eps[:], 1e-5)
            half = cpool.tile([P, 1], F32)
            nc.gpsimd.memset(half[:], 0.5)

            # ---- software-pipelined emission ----
            # Engines execute their instruction streams in order, so the
            # emission order below is skewed across super-tiles: every
            # iteration hands each engine work whose inputs were produced in
            # earlier iterations. Stage lags (loop index i):
            #   s0  load   (i)    : xT/ed DMA, sel one-hot builds (DVE)
            #   s1  agg    (i-1)  : 8*K chunk matmuls (PE)
            #   s2  aggS   (i-2)  : PSUM->SBUF copy (DVE)
            #   aL  layer head    : seed+z matmuls (PE), zc copy (DVE),
            #                       sq (POOL);       L1@i-3, L2@i-5, L3@i-7
            #   bL  layer tail    : var matmuls (PE), ln/exp (ACT), zn (POOL),
            #                       ez/h (ACT);      L1@i-4, L2@i-6, L3@i-8
            #   s6  store  (i-9)
            S = [dict() for _ in range(n_g)]

            def s0(g):
                off, w = groups[g]
                nt = w // P
                t0 = off // P
                d = S[g]
                xT = xpool.tile([P, GW], BF16, tag="xT")
                nc.sync.dma_start(out=xT[:, :w], in_=xt_h[:, off : off + w])
                d["xT"] = xT
                # one batched DMA per group for the edge payload and the
                # host-prebuilt one-hot sel (same layout)
                edg = epool.tile([P, 2 * TPS * K * P], BF16, tag="ed")
                nc.sync.dma_start(
                    out=edg[:, : nt * K * P].rearrange(
                        "p (t c) -> p t c", t=nt
                    ),
                    in_=edges_h[t0 * P : (t0 + nt) * P, :].rearrange(
                        "(t p) c -> p t c", t=nt
                    ),
                )
                selg = selpool.tile([P, 2 * TPS * K * P], BF16, tag="sel")
                nc.sync.dma_start(
                    out=selg[:, : nt * K * P].rearrange(
                        "p (t c) -> p t c", t=nt
                    ),
                    in_=selh_h[t0 * P : (t0 + nt) * P, :].rearrange(
                        "(t p) c -> p t c", t=nt
                    ),
                )
                d["edg"], d["selg"] = edg, selg

            def s1(g):
                off, w = groups[g]
                d = S[g]
                agg = pagg.tile([P, GW], F32, tag="agg")
                d["agg"] = agg
                for tg in range(w // P):
                    for k in range(K):
                        c0 = (tg * K + k) * P
                        nc.tensor.matmul(
                            out=agg[:, tg * P : (tg + 1) * P],
                            lhsT=d["edg"][:, c0 : c0 + P],
                            rhs=d["selg"][:, c0 : c0 + P],
                            start=(k == 0),
                            stop=(k == K - 1),
                        )

            def s2(g):
                off, w = groups[g]
                d = S[g]
                aggS = apool.tile([P, GW], BF16, tag="aggS")
                nc.vector.tensor_copy(aggS[:, :w], d["agg"][:, :w])
                d["aggS"] = aggS

            def a_stage(g, L):
                off, w = groups[g]
                d = S[g]
                z = psz.tile([P, GW], F32, tag="z")
                for h0 in range(0, w, SN):
                    hs = slice(h0, h0 + SN)
                    nc.tensor.matmul(
                        out=z[:, hs], lhsT=brow[L][:], rhs=ones_row[:],
                        start=True, stop=False,
                    )
                    if L == 1:
                        nc.tensor.matmul(
                            out=z[:, hs], lhsT=W["w1a"][:], rhs=d["xT"][:, hs],
                            start=False, stop=False,
                        )
                        nc.tensor.matmul(
                            out=z[:, hs], lhsT=W["w1b"][:],
                            rhs=d["aggS"][:, hs],
                            start=False, stop=True,
                        )
                    else:
                        nc.tensor.matmul(
                            out=z[:, hs], lhsT=W[f"w{L}"][:],
                            rhs=d[f"h{L - 1}"][:, hs],
                            start=False, stop=True,
                        )
                # PSUM has one DVE read port: copy z to SBUF bf16, square the
                # copy on DVE (all-bf16 SBUF operands hit the 2x mode).
                zc = sqpool.tile([P, GW], BF16, tag="zc")
                nc.vector.tensor_copy(zc[:, :w], z[:, :w])
                sq = sqpool.tile([P, GW], BF16, tag="sq")
                nc.vector.tensor_tensor(
                    sq[:, :w], zc[:, :w], zc[:, :w], op=mybir.AluOpType.mult
                )
                d[f"zc{L}"], d[f"sq{L}"] = zc, sq

            def b_stage(g, L):
                off, w = groups[g]
                d = S[g]
                var = psvar.tile([P, GW], F32, tag="var")
                for h0 in range(0, w, SN):
                    hs = slice(h0, h0 + SN)
                    nc.tensor.matmul(
                        out=var[:, hs], lhsT=ones_rep[:],
                        rhs=d[f"sq{L}"][:, hs],
                        start=True, stop=True,
                    )
                # rsig = exp(-0.5 * ln(var/H + eps)) broadcast over h
                lnv = rspool.tile([P, GW], BF16, tag="lnv")
                nc.scalar.activation(
                    lnv[:, :w], var[:, :w], mybir.ActivationFunctionType.Ln,
                    bias=eps[:, 0:1], scale=1.0 / H,
                )
                rsig = rspool.tile([P, GW], BF16, tag="rsig")
                nc.scalar.activation(
                    rsig[:, :w], lnv[:, :w], mybir.ActivationFunctionType.Exp,
                    scale=-0.5,
                )
                zn = znpool.tile([P, GW], BF16, tag="zn")
                nc.gpsimd.tensor_tensor(
                    zn[:, :w], d[f"zc{L}"][:, :w], rsig[:, :w],
                    op=mybir.AluOpType.mult,
                )
                # ssp(y) = ln(0.5*exp(y) + 0.5), y = g*zn + be; includes the
                # -log2 shift. |zn| <= sqrt(127) so exp cannot overflow.
                ez = ezpool.tile([P, GW], BF16, tag="ez")
                nc.scalar.activation(
                    ez[:, :w], zn[:, :w], mybir.ActivationFunctionType.Exp,
                    bias=V[f"be{L}"], scale=V[f"g{L}"],
                )
                hT = hpool.tile([P, GW], BF16, tag="hT")
                nc.scalar.activation(
                    hT[:, :w], ez[:, :w], mybir.ActivationFunctionType.Ln,
                    bias=half[:, 0:1], scale=0.5,
                )
                d[f"h{L}"] = hT

            def s6(g):
                off, w = groups[g]
                nc.sync.dma_start(
                    out=out_h[:, off : off + w], in_=S[g]["h3"][:, :w]
                )

            def run(fn, j, *args):
                if 0 <= j < n_g:
                    fn(j, *args)

            for i in range(n_g + 10):
                # aggS copy first (frees the agg PSUM bank), then layer
                # tails: their var matmuls and ACT chains are ready at
                # iteration start
                run(s2, i - 2)
                run(b_stage, i - 4, 1)
                run(b_stage, i - 6, 2)
                run(s0, i)
                run(s1, i - 1)
                run(b_stage, i - 8, 3)
                run(a_stage, i - 3, 1)
                run(a_stage, i - 5, 2)
                run(a_stage, i - 7, 3)
                run(s6, i - 9)

    if not nc.is_finalized():
        nc.finalize()
    return nc


def kernel(
    x, edge_index, edge_attr,
    W1, b1, g1, be1, W2, b2, g2, be2, W3, b3, g3, be3,
):
    global LAST_RESULT
    W1 = np.asarray(W1, np.float32)
    W2 = np.asarray(W2, np.float32)
    W3 = np.asarray(W3, np.float32)
    b1 = np.asarray(b1, np.float32)
    b2 = np.asarray(b2, np.float32)
    b3 = np.asarray(b3, np.float32)

    # Fold the LayerNorm mean into weights/biases: W' = W - rowmean, so the
    # matmul output is exactly mean-centered over the hidden dim.
    W1c = W1 - W1.mean(axis=1, keepdims=True)
    W2c = W2 - W2.mean(axis=1, keepdims=True)
    W3c = W3 - W3.mean(axis=1, keepdims=True)
    b1c = b1 - b1.mean()
    b2c = b2 - b2.mean()
    b3c = b3 - b3.mean()

    K, per_core = _host_prep(x, edge_index, edge_attr)
    nc = _build_program(K)

    vecs = np.stack(
        [np.asarray(v, np.float32) for v in (g1, g2, g3, be1, be2, be3)],
        axis=1,
    )  # [128, 6], column order must match VIDX in _build_program
    shared = {
        "w1a": np.ascontiguousarray(W1c[:P]).astype(ml_dtypes.bfloat16),
        "w1b": np.ascontiguousarray(W1c[P:]).astype(ml_dtypes.bfloat16),
        "w2": W2c.astype(ml_dtypes.bfloat16),
        "w3": W3c.astype(ml_dtypes.bfloat16),
        "b1": b1c.reshape(1, P).astype(ml_dtypes.bfloat16),
        "b2": b2c.reshape(1, P).astype(ml_dtypes.bfloat16),
        "b3": b3c.reshape(1, P).astype(ml_dtypes.bfloat16),
        "vecs": np.ascontiguousarray(vecs),
        "iota": np.ascontiguousarray(
            np.broadcast_to(
                np.tile(np.arange(P, dtype=np.float32), K), (P, K * P)
            )
        ),
    }
    in_maps = [
        {"edges": pay_c, "cols": col_c, "selh": sel_c, "xt": xt_c, **shared}
        for (pay_c, col_c, sel_c, xt_c) in per_core
    ]

    trace = bool(int(os.environ.get("KERNEL_TRACE", "0")))
    res = run_bass_kernel_spmd(nc, in_maps, core_ids=list(range(NC)), trace=trace)
    LAST_RESULT = res

    out = np.concatenate(
        [np.asarray(r["out"], dtype=np.float32).T for r in res.results], axis=0
    )
    return np.ascontiguousarray(out[:N])


# revision 30
# speedup vs baseline: 1.9128x; 1.0043x over previous
"""Trainium2 Bass kernel for nn_NodeModel (GNN message passing + 3-layer node MLP).

Strategy (node-parallel, 8 cores), v2 — transpose-free [h, node] dataflow:
  - Host: sort edges by destination node, bucket into 128-node tiles, pad each
    tile's edge list to K chunks of 128 edges. Nodes sharded contiguously
    across 8 cores (12800 padded nodes each, 25 super-tiles of 512).
  - LayerNorm mean is folded into the weights on host (W' = W - rowmean(W),
    b' = b - mean(b)) so every matmul output is already mean-centered.
  - Per 512-node super-tile on device:
      * aggregation: one-hot sel built by tensor_scalar(iota == col) on
        DVE/GPSIMD, chunk matmuls accumulate aggT[h, n] in PSUM.
      * per layer: bias seeded into PSUM by a rank-1 matmul (b'_row x ones),
        z_c = W'.T @ aT accumulated on top; var broadcast to all partitions
        via an all-ones stationary matmul of sq = z_c^2; rsig = exp(-.5*ln(.));
        zn = z_c * rsig; ssp via two ACT ops Exp(g*x+be), Ln(.5x+.5).
    Everything stays [h, node]; no PE transposes, no bn_stats.
  - ssp's -log2 appears exactly as ln(0.5 e^y + 0.5); output written bf16.
"""

import os
import sys

import numpy as np

sys.path.insert(0, "/opt/trn_rl_repo")

import bass_rust as _bass_rust
import ml_dtypes

from concourse import bacc, bass, hw_specs, mybir
from concourse import tile as tile_mod
from concourse.bass_utils import run_bass_kernel_spmd


class _Bacc(bacc.Bacc):
    """Bacc with the ACT table chooser pinned to the single function set
    that holds Ln+Exp+Copy+Identity. The default greedy chooser alternates
    between per-func sets, costing a ~1.3us ACT_TABLE_LOAD per switch."""

    def insert_act_table_loads(self):
        has_activation = any(
            isinstance(i, mybir.InstActivation)
            for b in self.main_func.blocks
            for i in b.instructions
        )
        if not has_activation:
            return
        keep = "natural_log_exp_and_others"
        tables = [
            (n, (s if n == keep else set()))
            for n, s in hw_specs.get_activation_tables(self.m.arch).items()
        ]
        _bass_rust.insert_act_table_loads(self, tables)


LOG2 = float(np.log(2.0))
N, E, H = 100000, 600000, 128
NC = 8
P = 128
SN = 512                 # nodes per super-tile
TPS = SN // P            # 128-node tiles per super-tile (4)
SPC = 25                 # super-tiles per core
TPC = SPC * TPS          # 128-node tiles per core (100)
NPC = TPC * P            # nodes per core (12800)
NPAD = NPC * NC          # padded node count (102400)
NT = NPAD // P           # total node tiles (800)

F32 = mybir.dt.float32
BF16 = mybir.dt.bfloat16

LAST_RESULT = None  # BassKernelResults of the most recent run (for profiling)


def _host_prep(x, edge_index, edge_attr):
    col = np.asarray(edge_index)[1].astype(np.int64)
    ea = np.ascontiguousarray(np.asarray(edge_attr, dtype=np.float32))
    order = np.argsort(col, kind="stable")
    col_s = col[order]
    tile_of = col_s >> 7
    counts = np.bincount(tile_of, minlength=NT)
    K = int(np.ceil(counts.max() / P))
    S = K * P
    starts = np.zeros(NT + 1, np.int64)
    starts[1:] = np.cumsum(counts)
    pos = np.arange(E) - starts[tile_of]
    slot = tile_of * S + pos
    slot_edge = np.zeros(NT * S, np.int64)
    slot_edge[slot] = order
    col_local = np.full(NT * S, 128.0, np.float32)
    col_local[slot] = (col_s & 127).astype(np.float32)
    payload = ea[slot_edge]  # [NT*S, H]

    # host-built one-hot sel, same row layout as the edge payload:
    # row (tile, edge-pos-in-chunk), col (chunk, local node)
    sel_full = np.zeros((NT * P, K * P), ml_dtypes.bfloat16)
    k_arr = (pos // P).astype(np.int64)
    e_arr = (pos % P).astype(np.int64)
    sel_full[tile_of * P + e_arr, k_arr * P + (col_s & 127)] = 1.0

    x_pad = np.zeros((NPAD, H), np.float32)
    x_pad[:N] = np.asarray(x, dtype=np.float32)

    per_core = []
    for c in range(NC):
        r0, r1 = c * TPC * S, (c + 1) * TPC * S
        # ed rows = edge position within chunk, cols = (tile-chunk, h)
        pay_c = np.ascontiguousarray(
            payload[r0:r1]
            .reshape(TPC, K, P, H)
            .transpose(0, 2, 1, 3)
            .reshape(TPC * P, K * H)
            .astype(ml_dtypes.bfloat16)
        )
        col_c = np.ascontiguousarray(
            col_local[r0:r1].reshape(TPC, K, P).transpose(2, 0, 1).reshape(P, TPC * K)
        )
        sel_c = np.ascontiguousarray(sel_full[c * TPC * P : (c + 1) * TPC * P])
        # xT: [h, node] per core
        xt_c = np.ascontiguousarray(
            x_pad[c * NPC : (c + 1) * NPC].T.astype(ml_dtypes.bfloat16)
        )
        per_core.append((pay_c, col_c, sel_c, xt_c))
    return K, per_core


def _build_program(K):
    # Bacc (not raw Bass): its compile pass splits multi-semaphore waits into
    # event-semaphore chains — walrus codegen allows only 1 wait per
    # instruction on this toolchain.
    nc = _Bacc("TRN2", target_bir_lowering=False, debug=False, num_devices=NC)

    edges_h = nc.dram_tensor("edges", [TPC * P, K * P], BF16, kind="ExternalInput")
    selh_h = nc.dram_tensor("selh", [TPC * P, K * P], BF16, kind="ExternalInput")
    cols_h = nc.dram_tensor("cols", [P, TPC * K], F32, kind="ExternalInput")
    xt_h = nc.dram_tensor("xt", [P, NPC], BF16, kind="ExternalInput")
    w_h = {
        name: nc.dram_tensor(name, [P, P], BF16, kind="ExternalInput")
        for name in ("w1a", "w1b", "w2", "w3")
    }
    # bias rows b1',b2',b3' (already mean-centered) as [1,128] for rank-1 seed
    brow_h = {
        i: nc.dram_tensor(f"b{i}", [1, P], BF16, kind="ExternalInput")
        for i in (1, 2, 3)
    }
    # g1..g3, be1..be3 packed as columns of one tensor
    vecs_h = nc.dram_tensor("vecs", [P, 6], F32, kind="ExternalInput")
    # iota ramp tiled K times along free dim, for wide one-hot builds
    iota_h = nc.dram_tensor("iota", [P, K * P], F32, kind="ExternalInput")
    out_h = nc.dram_tensor("out", [P, NPC], BF16, kind="ExternalOutput")
    VIDX = {n: i for i, n in enumerate(("g1", "g2", "g3", "be1", "be2", "be3"))}

    # groups of 2 super-tiles (1024 nodes): matmuls stay 512-wide (PSUM bank
    # limit), every elementwise/ACT op runs once per group at full width
    GW = 2 * SN
    sel_ship = int(os.environ.get("KERNEL_SELH", "8"))  # of 8 tiles per group
    n_st = int(os.environ.get("KERNEL_SPC", str(SPC)))
    groups = []
    off = 0
    while off < n_st * SN:
        w = min(GW, n_st * SN - off)
        groups.append((off, w))
        off += w
    n_g = len(groups)

    with tile_mod.TileContext(nc) as tc:
        with (
            tc.tile_pool(name="const", bufs=1) as cpool,
            tc.tile_pool(name="edges", bufs=3) as epool,
            tc.tile_pool(name="xin", bufs=5) as xpool,
            tc.tile_pool(name="sel", bufs=3) as selpool,
            tc.tile_pool(name="aggs", bufs=3) as apool,
            tc.tile_pool(name="sq", bufs=7) as sqpool,
            tc.tile_pool(name="rs", bufs=3) as rspool,
            tc.tile_pool(name="zn", bufs=3) as znpool,
            tc.tile_pool(name="ez", bufs=3) as ezpool,
            tc.tile_pool(name="hout", bufs=7) as hpool,
            # PSUM (8 banks), group tiles are 2 banks each: 1 agg + 2 z + 1 var
            tc.tile_pool(name="psagg", bufs=1, space="PSUM") as pagg,
            tc.tile_pool(name="psz", bufs=2, space="PSUM") as psz,
            tc.tile_pool(name="psvar", bufs=1, space="PSUM") as psvar,
        ):
            iota = cpool.tile_from(iota_h[:])
            cols = cpool.tile_from(cols_h[:])
            W = {k: cpool.tile_from(h[:], name=f"w_{k}") for k, h in w_h.items()}
            brow = {i: cpool.tile_from(h[:], name=f"b_{i}") for i, h in brow_h.items()}
            vecs = cpool.tile_from(vecs_h[:])
            V = {n: vecs[:, i : i + 1] for n, i in VIDX.items()}
            ones_rep = cpool.tile([P, P], BF16)
            nc.gpsimd.memset(ones_rep[:], 1.0)
            ones_row = cpool.tile([1, SN], BF16)
            nc.gpsimd.memset(ones_row[:], 1.0)
            eps = cpool.tile([P, 1], F32)
            nc.gpsimd.memset(eps[:], 1e-5)
            half = cpool.tile([P, 1], F32)
            nc.gpsimd.memset(half[:], 0.5)

            # ---- software-pipelined emission ----
            # Engines execute their instruction streams in order, so the
            # emission order below is skewed across super-tiles: every
            # iteration hands each engine work whose inputs were produced in
            # earlier iterations. Stage lags (loop index i):
            #   s0  load   (i)    : xT/ed DMA, sel one-hot builds (DVE)
            #   s1  agg    (i-1)  : 8*K chunk matmuls (PE)
            #   s2  aggS   (i-2)  : PSUM->SBUF copy (DVE)
            #   aL  layer head    : seed+z matmuls (PE), zc copy (DVE),
            #                       sq (POOL);       L1@i-3, L2@i-5, L3@i-7
            #   bL  layer tail    : var matmuls (PE), ln/exp (ACT), zn (POOL),
            #                       ez/h (ACT);      L1@i-4, L2@i-6, L3@i-8
            #   s6  store  (i-9)
            S = [dict() for _ in range(n_g)]

            def s0(g):
                off, w = groups[g]
                nt = w // P
                t0 = off // P
                d = S[g]
                xT = xpool.tile([P, GW], BF16, tag="xT")
                nc.sync.dma_start(out=xT[:, :w], in_=xt_h[:, off : off + w])
                d["xT"] = xT
                # one batched DMA per group for the edge payload and the
                # host-prebuilt one-hot sel (same layout)
                edg = epool.tile([P, 2 * TPS * K * P], BF16, tag="ed")
                nc.sync.dma_start(
                    out=edg[:, : nt * K * P].rearrange(
                        "p (t c) -> p t c", t=nt
                    ),
                    in_=edges_h[t0 * P : (t0 + nt) * P, :].rearrange(
                        "(t p) c -> p t c", t=nt
                    ),
                )
                selg = selpool.tile([P, 2 * TPS * K * P], BF16, tag="sel")
                nc.sync.dma_start(
                    out=selg[:, : nt * K * P].rearrange(
                        "p (t c) -> p t c", t=nt
                    ),
                    in_=selh_h[t0 * P : (t0 + nt) * P, :].rearrange(
                        "(t p) c -> p t c", t=nt
                    ),
                )
                d["edg"], d["selg"] = edg, selg

            def s1(g):
                off, w = groups[g]
                d = S[g]
                agg = pagg.tile([P, GW], F32, tag="agg")
                d["agg"] = agg
                for tg in range(w // P):
                    for k in range(K):
                        c0 = (tg * K + k) * P
                        nc.tensor.matmul(
                            out=agg[:, tg * P : (tg + 1) * P],
                            lhsT=d["edg"][:, c0 : c0 + P],
                            rhs=d["selg"][:, c0 : c0 + P],
                            start=(k == 0),
                            stop=(k == K - 1),
                        )

            def s2(g):
                off, w = groups[g]
                d = S[g]
                aggS = apool.tile([P, GW], BF16, tag="aggS")
                nc.vector.tensor_copy(aggS[:, :w], d["agg"][:, :w])
                d["aggS"] = aggS

            def a_stage(g, L):
                off, w = groups[g]
                d = S[g]
                z = psz.tile([P, GW], F32, tag="z")
                for h0 in range(0, w, SN):
                    hs = slice(h0, h0 + SN)
                    nc.tensor.matmul(
                        out=z[:, hs], lhsT=brow[L][:], rhs=ones_row[:],
                        start=True, stop=False,
                    )
                    if L == 1:
                        nc.tensor.matmul(
                            out=z[:, hs], lhsT=W["w1a"][:], rhs=d["xT"][:, hs],
                            start=False, stop=False,
                        )
                        nc.tensor.matmul(
                            out=z[:, hs], lhsT=W["w1b"][:],
                            rhs=d["aggS"][:, hs],
                            start=False, stop=True,
                        )
                    else:
                        nc.tensor.matmul(
                            out=z[:, hs], lhsT=W[f"w{L}"][:],
                            rhs=d[f"h{L - 1}"][:, hs],
                            start=False, stop=True,
                        )
                # PSUM has one DVE read port: copy z to SBUF bf16, square the
                # copy on GPSIMD — sq is consumed one iteration later, so
                # POOL's slower rate stays off the critical chain.
                zc = sqpool.tile([P, GW], BF16, tag="zc")
                nc.vector.tensor_copy(zc[:, :w], z[:, :w])
                sq = sqpool.tile([P, GW], BF16, tag="sq")
                nc.gpsimd.tensor_tensor(
                    sq[:, :w], zc[:, :w], zc[:, :w], op=mybir.AluOpType.mult
                )
                d[f"zc{L}"], d[f"sq{L}"] = zc, sq

            def b_stage(g, L):
                off, w = groups[g]
                d = S[g]
                var = psvar.tile([P, GW], F32, tag="var")
                for h0 in range(0, w, SN):
                    hs = slice(h0, h0 + SN)
                    nc.tensor.matmul(
                        out=var[:, hs], lhsT=ones_rep[:],
                        rhs=d[f"sq{L}"][:, hs],
                        start=True, stop=True,
                    )
                # rsig = exp(-0.5 * ln(var/H + eps)) broadcast over h
                lnv = rspool.tile([P, GW], BF16, tag="lnv")
                nc.scalar.activation(
                    lnv[:, :w], var[:, :w], mybir.ActivationFunctionType.Ln,
                    bias=eps[:, 0:1], scale=1.0 / H,
                )
                rsig = rspool.tile([P, GW], BF16, tag="rsig")
                nc.scalar.activation(
                    rsig[:, :w], lnv[:, :w], mybir.ActivationFunctionType.Exp,
                    scale=-0.5,
                )
                # zn on DVE (2x bf16 mode): it sits on the critical chain
                # rsig -> zn -> ez, where POOL's 1x rate would stall ACT
                zn = znpool.tile([P, GW], BF16, tag="zn")
                nc.vector.tensor_tensor(
                    zn[:, :w], d[f"zc{L}"][:, :w], rsig[:, :w],
                    op=mybir.AluOpType.mult,
                )
                # ssp(y) = ln(0.5*exp(y) + 0.5), y = g*zn + be; includes the
                # -log2 shift. |zn| <= sqrt(127) so exp cannot overflow.
                ez = ezpool.tile([P, GW], BF16, tag="ez")
                nc.scalar.activation(
                    ez[:, :w], zn[:, :w], mybir.ActivationFunctionType.Exp,
                    bias=V[f"be{L}"], scale=V[f"g{L}"],
                )
                hT = hpool.tile([P, GW], BF16, tag="hT")
                nc.scalar.activation(
                    hT[:, :w], ez[:, :w], mybir.ActivationFunctionType.Ln,
                    bias=half[:, 0:1], scale=0.5,
                )
                d[f"h{L}"] = hT

            def s6(g):
                off, w = groups[g]
                nc.sync.dma_start(
                    out=out_h[:, off : off + w], in_=S[g]["h3"][:, :w]
                )

            def run(fn, j, *args):
                if 0 <= j < n_g:
                    fn(j, *args)

            for i in range(n_g + 10):
                # aggS copy first (frees the agg PSUM bank), then layer
                # tails: their var matmuls and ACT chains are ready at
                # iteration start
                run(s2, i - 2)
                run(b_stage, i - 4, 1)
                run(b_stage, i - 6, 2)
                run(s0, i)
                run(s1, i - 1)
                run(b_stage, i - 8, 3)
                run(a_stage, i - 3, 1)
                run(a_stage, i - 5, 2)
                run(a_stage, i - 7, 3)
                run(s6, i - 9)

    if not nc.is_finalized():
        nc.finalize()
    return nc


def kernel(
    x, edge_index, edge_attr,
    W1, b1, g1, be1, W2, b2, g2, be2, W3, b3, g3, be3,
):
    global LAST_RESULT
    W1 = np.asarray(W1, np.float32)
    W2 = np.asarray(W2, np.float32)
    W3 = np.asarray(W3, np.float32)
    b1 = np.asarray(b1, np.float32)
    b2 = np.asarray(b2, np.float32)
    b3 = np.asarray(b3, np.float32)

    # Fold the LayerNorm mean into weights/biases: W' = W - rowmean, so the
    # matmul output is exactly mean-centered over the hidden dim.
    W1c = W1 - W1.mean(axis=1, keepdims=True)
    W2c = W2 - W2.mean(axis=1, keepdims=True)
    W3c = W3 - W3.mean(axis=1, keepdims=True)
    b1c = b1 - b1.mean()
    b2c = b2 - b2.mean()
    b3c = b3 - b3.mean()

    K, per_core = _host_prep(x, edge_index, edge_attr)
    nc = _build_program(K)

    vecs = np.stack(
        [np.asarray(v, np.float32) for v in (g1, g2, g3, be1, be2, be3)],
        axis=1,
    )  # [128, 6], column order must match VIDX in _build_program
    shared = {
        "w1a": np.ascontiguousarray(W1c[:P]).astype(ml_dtypes.bfloat16),
        "w1b": np.ascontiguousarray(W1c[P:]).astype(ml_dtypes.bfloat16),
        "w2": W2c.astype(ml_dtypes.bfloat16),
        "w3": W3c.astype(ml_dtypes.bfloat16),
        "b1": b1c.reshape(1, P).astype(ml_dtypes.bfloat16),
        "b2": b2c.reshape(1, P).astype(ml_dtypes.bfloat16),
        "b3": b3c.reshape(1, P).astype(ml_dtypes.bfloat16),
        "vecs": np.ascontiguousarray(vecs),
        "iota": np.ascontiguousarray(
            np.broadcast_to(
                np.tile(np.arange(P, dtype=np.float32), K), (P, K * P)
            )
        ),
    }
    in_maps = [
        {"edges": pay_c, "cols": col_c, "selh": sel_c, "xt": xt_c, **shared}
        for (pay_c, col_c, sel_c, xt_c) in per_core
    ]

    trace = bool(int(os.environ.get("KERNEL_TRACE", "0")))
    res = run_bass_kernel_spmd(nc, in_maps, core_ids=list(range(NC)), trace=trace)
    LAST_RESULT = res

    out = np.concatenate(
        [np.asarray(r["out"], dtype=np.float32).T for r in res.results], axis=0
    )
    return np.ascontiguousarray(out[:N])


# revision 31
# speedup vs baseline: 1.9510x; 1.0200x over previous
"""Trainium2 Bass kernel for nn_NodeModel (GNN message passing + 3-layer node MLP).

Strategy (node-parallel, 8 cores), v2 — transpose-free [h, node] dataflow:
  - Host: sort edges by destination node, bucket into 128-node tiles, pad each
    tile's edge list to K chunks of 128 edges. Nodes sharded contiguously
    across 8 cores (12800 padded nodes each, 25 super-tiles of 512).
  - LayerNorm mean is folded into the weights on host (W' = W - rowmean(W),
    b' = b - mean(b)) so every matmul output is already mean-centered.
  - Per 512-node super-tile on device:
      * aggregation: one-hot sel built by tensor_scalar(iota == col) on
        DVE/GPSIMD, chunk matmuls accumulate aggT[h, n] in PSUM.
      * per layer: bias seeded into PSUM by a rank-1 matmul (b'_row x ones),
        z_c = W'.T @ aT accumulated on top; var broadcast to all partitions
        via an all-ones stationary matmul of sq = z_c^2; rsig = exp(-.5*ln(.));
        zn = z_c * rsig; ssp via two ACT ops Exp(g*x+be), Ln(.5x+.5).
    Everything stays [h, node]; no PE transposes, no bn_stats.
  - ssp's -log2 appears exactly as ln(0.5 e^y + 0.5); output written bf16.
"""

import os
import sys

import numpy as np

sys.path.insert(0, "/opt/trn_rl_repo")

import bass_rust as _bass_rust
import ml_dtypes

from concourse import bacc, bass, hw_specs, mybir
from concourse import tile as tile_mod
from concourse.bass_utils import run_bass_kernel_spmd


class _Bacc(bacc.Bacc):
    """Bacc with the ACT table chooser pinned to the single function set
    that holds Ln+Exp+Copy+Identity. The default greedy chooser alternates
    between per-func sets, costing a ~1.3us ACT_TABLE_LOAD per switch."""

    def insert_act_table_loads(self):
        has_activation = any(
            isinstance(i, mybir.InstActivation)
            for b in self.main_func.blocks
            for i in b.instructions
        )
        if not has_activation:
            return
        keep = "natural_log_exp_and_others"
        tables = [
            (n, (s if n == keep else set()))
            for n, s in hw_specs.get_activation_tables(self.m.arch).items()
        ]
        _bass_rust.insert_act_table_loads(self, tables)


LOG2 = float(np.log(2.0))
N, E, H = 100000, 600000, 128
NC = 8
P = 128
SN = 512                 # nodes per super-tile
TPS = SN // P            # 128-node tiles per super-tile (4)
SPC = 25                 # super-tiles per core
TPC = SPC * TPS          # 128-node tiles per core (100)
NPC = TPC * P            # nodes per core (12800)
NPAD = NPC * NC          # padded node count (102400)
NT = NPAD // P           # total node tiles (800)

F32 = mybir.dt.float32
BF16 = mybir.dt.bfloat16

LAST_RESULT = None  # BassKernelResults of the most recent run (for profiling)


def _host_prep(x, edge_index, edge_attr):
    col = np.asarray(edge_index)[1].astype(np.int64)
    ea = np.ascontiguousarray(np.asarray(edge_attr, dtype=np.float32))
    order = np.argsort(col, kind="stable")
    col_s = col[order]
    tile_of = col_s >> 7
    counts = np.bincount(tile_of, minlength=NT)
    K = int(np.ceil(counts.max() / P))
    S = K * P
    starts = np.zeros(NT + 1, np.int64)
    starts[1:] = np.cumsum(counts)
    pos = np.arange(E) - starts[tile_of]
    slot = tile_of * S + pos
    slot_edge = np.zeros(NT * S, np.int64)
    slot_edge[slot] = order
    col_local = np.full(NT * S, 128.0, np.float32)
    col_local[slot] = (col_s & 127).astype(np.float32)
    payload = ea[slot_edge]  # [NT*S, H]

    # host-built one-hot sel, same row layout as the edge payload:
    # row (tile, edge-pos-in-chunk), col (chunk, local node)
    sel_full = np.zeros((NT * P, K * P), ml_dtypes.bfloat16)
    k_arr = (pos // P).astype(np.int64)
    e_arr = (pos % P).astype(np.int64)
    sel_full[tile_of * P + e_arr, k_arr * P + (col_s & 127)] = 1.0

    x_pad = np.zeros((NPAD, H), np.float32)
    x_pad[:N] = np.asarray(x, dtype=np.float32)

    per_core = []
    for c in range(NC):
        r0, r1 = c * TPC * S, (c + 1) * TPC * S
        # ed rows = edge position within chunk, cols = (tile-chunk, h)
        pay_c = np.ascontiguousarray(
            payload[r0:r1]
            .reshape(TPC, K, P, H)
            .transpose(0, 2, 1, 3)
            .reshape(TPC * P, K * H)
            .astype(ml_dtypes.bfloat16)
        )
        col_c = np.ascontiguousarray(
            col_local[r0:r1].reshape(TPC, K, P).transpose(2, 0, 1).reshape(P, TPC * K)
        )
        sel_c = np.ascontiguousarray(sel_full[c * TPC * P : (c + 1) * TPC * P])
        # xT: [h, node] per core
        xt_c = np.ascontiguousarray(
            x_pad[c * NPC : (c + 1) * NPC].T.astype(ml_dtypes.bfloat16)
        )
        per_core.append((pay_c, col_c, sel_c, xt_c))
    return K, per_core


def _build_program(K):
    # Bacc (not raw Bass): its compile pass splits multi-semaphore waits into
    # event-semaphore chains — walrus codegen allows only 1 wait per
    # instruction on this toolchain.
    nc = _Bacc("TRN2", target_bir_lowering=False, debug=False, num_devices=NC)

    edges_h = nc.dram_tensor("edges", [TPC * P, K * P], BF16, kind="ExternalInput")
    selh_h = nc.dram_tensor("selh", [TPC * P, K * P], BF16, kind="ExternalInput")
    cols_h = nc.dram_tensor("cols", [P, TPC * K], F32, kind="ExternalInput")
    xt_h = nc.dram_tensor("xt", [P, NPC], BF16, kind="ExternalInput")
    w_h = {
        name: nc.dram_tensor(name, [P, P], BF16, kind="ExternalInput")
        for name in ("w1a", "w1b", "w2", "w3")
    }
    # bias rows b1',b2',b3' (already mean-centered) as [1,128] for rank-1 seed
    brow_h = {
        i: nc.dram_tensor(f"b{i}", [1, P], BF16, kind="ExternalInput")
        for i in (1, 2, 3)
    }
    # g1..g3, be1..be3 packed as columns of one tensor
    vecs_h = nc.dram_tensor("vecs", [P, 6], F32, kind="ExternalInput")
    # iota ramp tiled K times along free dim, for wide one-hot builds
    iota_h = nc.dram_tensor("iota", [P, K * P], F32, kind="ExternalInput")
    out_h = nc.dram_tensor("out", [P, NPC], BF16, kind="ExternalOutput")
    VIDX = {n: i for i, n in enumerate(("g1", "g2", "g3", "be1", "be2", "be3"))}

    # groups of 2 super-tiles (1024 nodes): matmuls stay 512-wide (PSUM bank
    # limit), every elementwise/ACT op runs once per group at full width
    GW = 2 * SN
    sel_ship = int(os.environ.get("KERNEL_SELH", "8"))  # of 8 tiles per group
    n_st = int(os.environ.get("KERNEL_SPC", str(SPC)))
    groups = []
    off = 0
    while off < n_st * SN:
        w = min(GW, n_st * SN - off)
        groups.append((off, w))
        off += w
    n_g = len(groups)

    with tile_mod.TileContext(nc) as tc:
        with (
            tc.tile_pool(name="const", bufs=1) as cpool,
            tc.tile_pool(name="edges", bufs=3) as epool,
            tc.tile_pool(name="xin", bufs=5) as xpool,
            tc.tile_pool(name="sel", bufs=3) as selpool,
            tc.tile_pool(name="aggs", bufs=3) as apool,
            tc.tile_pool(name="sq", bufs=7) as sqpool,
            tc.tile_pool(name="rs", bufs=3) as rspool,
            tc.tile_pool(name="zn", bufs=3) as znpool,
            tc.tile_pool(name="ez", bufs=3) as ezpool,
            tc.tile_pool(name="hout", bufs=7) as hpool,
            # PSUM (8 banks), group tiles are 2 banks each: 1 agg + 2 z + 1 var
            tc.tile_pool(name="psagg", bufs=1, space="PSUM") as pagg,
            tc.tile_pool(name="psz", bufs=2, space="PSUM") as psz,
            tc.tile_pool(name="psvar", bufs=1, space="PSUM") as psvar,
        ):
            iota = cpool.tile_from(iota_h[:])
            cols = cpool.tile_from(cols_h[:])
            W = {k: cpool.tile_from(h[:], name=f"w_{k}") for k, h in w_h.items()}
            brow = {i: cpool.tile_from(h[:], name=f"b_{i}") for i, h in brow_h.items()}
            vecs = cpool.tile_from(vecs_h[:])
            V = {n: vecs[:, i : i + 1] for n, i in VIDX.items()}
            ones_rep = cpool.tile([P, P], BF16)
            nc.gpsimd.memset(ones_rep[:], 1.0)
            ones_row = cpool.tile([1, SN], BF16)
            nc.gpsimd.memset(ones_row[:], 1.0)
            eps = cpool.tile([P, 1], F32)
            nc.gpsimd.memset(eps[:], 1e-5)
            half = cpool.tile([P, 1], F32)
            nc.gpsimd.memset(half[:], 0.5)

            # ---- software-pipelined emission ----
            # Engines execute their instruction streams in order, so the
            # emission order below is skewed across super-tiles: every
            # iteration hands each engine work whose inputs were produced in
            # earlier iterations. Stage lags (loop index i):
            #   s0  load   (i)    : xT/ed DMA, sel one-hot builds (DVE)
            #   s1  agg    (i-1)  : 8*K chunk matmuls (PE)
            #   s2  aggS   (i-2)  : PSUM->SBUF copy (DVE)
            #   aL  layer head    : seed+z matmuls (PE), zc copy (DVE),
            #                       sq (POOL);       L1@i-3, L2@i-5, L3@i-7
            #   bL  layer tail    : var matmuls (PE), ln/exp (ACT), zn (POOL),
            #                       ez/h (ACT);      L1@i-4, L2@i-6, L3@i-8
            #   s6  store  (i-9)
            S = [dict() for _ in range(n_g)]

            def s0(g):
                off, w = groups[g]
                nt = w // P
                t0 = off // P
                d = S[g]
                xT = xpool.tile([P, GW], BF16, tag="xT")
                nc.sync.dma_start(out=xT[:, :w], in_=xt_h[:, off : off + w])
                d["xT"] = xT
                # one batched DMA per group for the edge payload and the
                # host-prebuilt one-hot sel (same layout)
                edg = epool.tile([P, 2 * TPS * K * P], BF16, tag="ed")
                nc.sync.dma_start(
                    out=edg[:, : nt * K * P].rearrange(
                        "p (t c) -> p t c", t=nt
                    ),
                    in_=edges_h[t0 * P : (t0 + nt) * P, :].rearrange(
                        "(t p) c -> p t c", t=nt
                    ),
                )
                selg = selpool.tile([P, 2 * TPS * K * P], BF16, tag="sel")
                nc.sync.dma_start(
                    out=selg[:, : nt * K * P].rearrange(
                        "p (t c) -> p t c", t=nt
                    ),
                    in_=selh_h[t0 * P : (t0 + nt) * P, :].rearrange(
                        "(t p) c -> p t c", t=nt
                    ),
                )
                d["edg"], d["selg"] = edg, selg

            def s1(g):
                off, w = groups[g]
                d = S[g]
                agg = pagg.tile([P, GW], F32, tag="agg")
                d["agg"] = agg
                for tg in range(w // P):
                    for k in range(K):
                        c0 = (tg * K + k) * P
                        nc.tensor.matmul(
                            out=agg[:, tg * P : (tg + 1) * P],
                            lhsT=d["edg"][:, c0 : c0 + P],
                            rhs=d["selg"][:, c0 : c0 + P],
                            start=(k == 0),
                            stop=(k == K - 1),
                        )

            def s2(g):
                off, w = groups[g]
                d = S[g]
                aggS = apool.tile([P, GW], BF16, tag="aggS")
                nc.vector.tensor_copy(aggS[:, :w], d["agg"][:, :w])
                d["aggS"] = aggS

            def a_stage(g, L):
                off, w = groups[g]
                d = S[g]
                z = psz.tile([P, GW], F32, tag="z")
                for h0 in range(0, w, SN):
                    hs = slice(h0, h0 + SN)
                    nc.tensor.matmul(
                        out=z[:, hs], lhsT=brow[L][:], rhs=ones_row[:],
                        start=True, stop=False,
                    )
                    if L == 1:
                        nc.tensor.matmul(
                            out=z[:, hs], lhsT=W["w1a"][:], rhs=d["xT"][:, hs],
                            start=False, stop=False,
                        )
                        nc.tensor.matmul(
                            out=z[:, hs], lhsT=W["w1b"][:],
                            rhs=d["aggS"][:, hs],
                            start=False, stop=True,
                        )
                    else:
                        nc.tensor.matmul(
                            out=z[:, hs], lhsT=W[f"w{L}"][:],
                            rhs=d[f"h{L - 1}"][:, hs],
                            start=False, stop=True,
                        )
                # PSUM has one DVE read port: copy z to SBUF bf16, square the
                # copy on GPSIMD — sq is consumed one iteration later, so
                # POOL's slower rate stays off the critical chain.
                zc = sqpool.tile([P, GW], BF16, tag="zc")
                nc.vector.tensor_copy(zc[:, :w], z[:, :w])
                sq = sqpool.tile([P, GW], BF16, tag="sq")
                nc.gpsimd.tensor_tensor(
                    sq[:, :w], zc[:, :w], zc[:, :w], op=mybir.AluOpType.mult
                )
                d[f"zc{L}"], d[f"sq{L}"] = zc, sq

            def b_stage(g, L):
                off, w = groups[g]
                d = S[g]
                var = psvar.tile([P, GW], F32, tag="var")
                for h0 in range(0, w, SN):
                    hs = slice(h0, h0 + SN)
                    nc.tensor.matmul(
                        out=var[:, hs], lhsT=ones_rep[:],
                        rhs=d[f"sq{L}"][:, hs],
                        start=True, stop=True,
                    )
                # rsig = exp(-0.5 * ln(var/H + eps)) broadcast over h
                lnv = rspool.tile([P, GW], BF16, tag="lnv")
                nc.scalar.activation(
                    lnv[:, :w], var[:, :w], mybir.ActivationFunctionType.Ln,
                    bias=eps[:, 0:1], scale=1.0 / H,
                )
                rsig = rspool.tile([P, GW], BF16, tag="rsig")
                nc.scalar.activation(
                    rsig[:, :w], lnv[:, :w], mybir.ActivationFunctionType.Exp,
                    scale=-0.5,
                )
                # zn on DVE (2x bf16 mode): it sits on the critical chain
                # rsig -> zn -> ez, where POOL's 1x rate would stall ACT
                zn = znpool.tile([P, GW], BF16, tag="zn")
                nc.vector.tensor_tensor(
                    zn[:, :w], d[f"zc{L}"][:, :w], rsig[:, :w],
                    op=mybir.AluOpType.mult,
                )
                # ssp(y) = ln(0.5*exp(y) + 0.5), y = g*zn + be; includes the
                # -log2 shift. |zn| <= sqrt(127) so exp cannot overflow.
                ez = ezpool.tile([P, GW], BF16, tag="ez")
                nc.scalar.activation(
                    ez[:, :w], zn[:, :w], mybir.ActivationFunctionType.Exp,
                    bias=V[f"be{L}"], scale=V[f"g{L}"],
                )
                hT = hpool.tile([P, GW], BF16, tag="hT")
                nc.scalar.activation(
                    hT[:, :w], ez[:, :w], mybir.ActivationFunctionType.Ln,
                    bias=half[:, 0:1], scale=0.5,
                )
                d[f"h{L}"] = hT

            def s6(g):
                off, w = groups[g]
                nc.sync.dma_start(
                    out=out_h[:, off : off + w], in_=S[g]["h3"][:, :w]
                )

            def run(fn, j, *args):
                if 0 <= j < n_g:
                    fn(j, *args)

            for i in range(n_g + 10):
                # aggS copy first (frees the agg PSUM bank), then layer
                # tails (their var matmuls and ACT chains are ready at
                # iteration start), then layer heads — z matmuls early in
                # the PE stream so zc/sq complete well before the next
                # iteration's var matmuls — and the agg matmuls fill the
                # PE back-half.
                run(s2, i - 2)
                run(b_stage, i - 4, 1)
                run(b_stage, i - 6, 2)
                run(a_stage, i - 3, 1)
                run(a_stage, i - 5, 2)
                run(a_stage, i - 7, 3)
                run(b_stage, i - 8, 3)
                run(s0, i)
                run(s1, i - 1)
                run(s6, i - 9)

    if not nc.is_finalized():
        nc.finalize()
    return nc


def kernel(
    x, edge_index, edge_attr,
    W1, b1, g1, be1, W2, b2, g2, be2, W3, b3, g3, be3,
):
    global LAST_RESULT
    W1 = np.asarray(W1, np.float32)
    W2 = np.asarray(W2, np.float32)
    W3 = np.asarray(W3, np.float32)
    b1 = np.asarray(b1, np.float32)
    b2 = np.asarray(b2, np.float32)
    b3 = np.asarray(b3, np.float32)

    # Fold the LayerNorm mean into weights/biases: W' = W - rowmean, so the
    # matmul output is exactly mean-centered over the hidden dim.
    W1c = W1 - W1.mean(axis=1, keepdims=True)
    W2c = W2 - W2.mean(axis=1, keepdims=True)
    W3c = W3 - W3.mean(axis=1, keepdims=True)
    b1c = b1 - b1.mean()
    b2c = b2 - b2.mean()
    b3c = b3 - b3.mean()

    K, per_core = _host_prep(x, edge_index, edge_attr)
    nc = _build_program(K)

    vecs = np.stack(
        [np.asarray(v, np.float32) for v in (g1, g2, g3, be1, be2, be3)],
        axis=1,
    )  # [128, 6], column order must match VIDX in _build_program
    shared = {
        "w1a": np.ascontiguousarray(W1c[:P]).astype(ml_dtypes.bfloat16),
        "w1b": np.ascontiguousarray(W1c[P:]).astype(ml_dtypes.bfloat16),
        "w2": W2c.astype(ml_dtypes.bfloat16),
        "w3": W3c.astype(ml_dtypes.bfloat16),
        "b1": b1c.reshape(1, P).astype(ml_dtypes.bfloat16),
        "b2": b2c.reshape(1, P).astype(ml_dtypes.bfloat16),
        "b3": b3c.reshape(1, P).astype(ml_dtypes.bfloat16),
        "vecs": np.ascontiguousarray(vecs),
        "iota": np.ascontiguousarray(
            np.broadcast_to(
                np.tile(np.arange(P, dtype=np.float32), K), (P, K * P)
            )
        ),
    }
    in_maps = [
        {"edges": pay_c, "cols": col_c, "selh": sel_c, "xt": xt_c, **shared}
        for (pay_c, col_c, sel_c, xt_c) in per_core
    ]

    trace = bool(int(os.environ.get("KERNEL_TRACE", "0")))
    res = run_bass_kernel_spmd(nc, in_maps, core_ids=list(range(NC)), trace=trace)
    LAST_RESULT = res

    out = np.concatenate(
        [np.asarray(r["out"], dtype=np.float32).T for r in res.results], axis=0
    )
    return np.ascontiguousarray(out[:N])
